# revision 19
# baseline (speedup 1.0000x reference)
"""Trainium2 Bass kernel for nn_MetaDiscreteTimeTrajTypeGRUNet.

Strategy (8 NeuronCores, SPMD):
  - GRU (2 layers, T=48, B=32) replicated on all cores (latency-bound, tiny).
    Wavefront schedule: layer 2 runs 4 steps behind layer 1.
  - FC + log_softmax vocab-parallel: each core owns 5000 output columns
    (+ every core computes the small 48-col time head; core 0's is used).
  - log_softmax without max-subtraction (logits are O(1) by construction):
    pass 1 computes sum(exp(x+b)) per row, one 6KB AllReduce combines the
    8 vocab shards, pass 2 recomputes the matmul (weights stay in SBUF)
    and writes x + b - log(Z) directly.
  - All matmuls in float32r (full PE rate at N>=256).

Row indexing on device is t-major: r = t*32 + b; chunk c = rows 128c..128c+127
(= timesteps 4c..4c+3, all 32 batch elements each). DRAM outputs are written
b-major via strided 3D access patterns.
"""
import numpy as np

B, T, E, TD = 32, 48, 128, 48
H, OUT, TRAJ = 256, 40000, 10
DIN = E + TD          # 176
G = 3 * H             # 768
NCORES = 8
VS = OUT // NCORES    # 5000 vocab columns per core
VTOT = VS + TD        # 5048 fc columns per core (vocab slice + time head)
R = B * T             # 1536
RC = R // 128         # 12 row chunks
NCH = 10              # vocab N-chunks per row chunk
NW = VS // NCH        # 500

_cache = {}


def _build(gru_bias_nz: bool, fcb_nz: bool):
    import concourse.bass as bass
    import concourse.mybir as mybir
    import concourse.tile as tile
    import concourse.bacc as bacc
    from concourse.masks import make_identity

    f32 = mybir.dt.float32
    f32r = mybir.dt.float32r
    i32 = mybir.dt.int32
    AF = mybir.ActivationFunctionType
    ALU = mybir.AluOpType
    AX = mybir.AxisListType

    nc = bacc.Bacc("TRN2", target_bir_lowering=False, debug=False,
                   num_devices=NCORES)

    # ---------------- I/O ----------------
    loc_tm = nc.dram_tensor("loc_tm", [128, RC], i32, kind="ExternalInput")
    tim_tm = nc.dram_tensor("tim_tm", [128, RC], i32, kind="ExternalInput")
    lab_ix = nc.dram_tensor("lab_ix", [2 * B, 1], i32, kind="ExternalInput")
    embed = nc.dram_tensor("embed", [OUT, E], f32, kind="ExternalInput")
    traj = nc.dram_tensor("traj", [2 * TRAJ, H], f32, kind="ExternalInput")
    wih0T_d = nc.dram_tensor("wih0T", [DIN, G], f32, kind="ExternalInput")
    wih1T_d = nc.dram_tensor("wih1T", [H, G], f32, kind="ExternalInput")
    whh0T_d = nc.dram_tensor("whh0T", [H, G], f32, kind="ExternalInput")
    whh1T_d = nc.dram_tensor("whh1T", [H, G], f32, kind="ExternalInput")
    fcwT_d = nc.dram_tensor("fcwT", [H, VTOT], f32, kind="ExternalInput")
    if gru_bias_nz:
        gb0_d = nc.dram_tensor("gb0", [2, G], f32, kind="ExternalInput")
        gb1_d = nc.dram_tensor("gb1", [2, G], f32, kind="ExternalInput")
    if fcb_nz:
        fcb_d = nc.dram_tensor("fcb", [1, VTOT], f32, kind="ExternalInput")

    out_loc = nc.dram_tensor("out_loc", [B, T, VS], f32, kind="ExternalOutput")
    out_tm = nc.dram_tensor("out_tm", [B, T, TD], f32, kind="ExternalOutput")

    with tile.TileContext(nc) as tc:
        with (
            tc.tile_pool(name="const", bufs=1) as cp,
            tc.tile_pool(name="dram", bufs=1, space="DRAM") as dp,
        ):
            # ---------- constants ----------
            idn = cp.tile([128, 128], f32, tag="idn", name="idn")
            make_identity(nc, idn[:])

            iota_i = cp.tile([128, TD], i32, tag="iota_i", name="iota_i")
            nc.gpsimd.iota(iota_i[:], pattern=[[1, TD]], base=0,
                           channel_multiplier=0)
            iota_f = cp.tile([128, TD], f32, tag="iota_f", name="iota_f")
            nc.vector.tensor_copy(iota_f[:], iota_i[:])

            loc_sb = cp.tile([128, RC], i32, tag="loc_sb", name="loc_sb")
            nc.sync.dma_start(loc_sb[:], loc_tm[:])
            tim_i = cp.tile([128, RC], i32, tag="tim_i", name="tim_i")
            nc.sync.dma_start(tim_i[:], tim_tm[:])
            tim_f = cp.tile([128, RC], f32, tag="tim_f", name="tim_f")
            nc.vector.tensor_copy(tim_f[:], tim_i[:])

            # weights -> SBUF as f32r (DMA to a staging tile, then a rounding
            # copy into the f32r tile the matmuls consume)
            with tc.tile_pool(name="stage", bufs=2) as stp:
                def load_w(dram_h, r0, r1, cols, tag):
                    st = stp.tile([r1 - r0, cols], f32, tag="wstage",
                                  name="wstage")
                    nc.sync.dma_start(st[:], dram_h[r0:r1, :])
                    t = cp.tile([r1 - r0, cols], f32r, tag=tag, name=tag)
                    nc.vector.tensor_copy(t[:], st[:])
                    return t

                wih0a = load_w(wih0T_d, 0, 128, G, "wih0a")
                wih0b = load_w(wih0T_d, 128, DIN, G, "wih0b")
                wih1a = load_w(wih1T_d, 0, 128, G, "wih1a")
                wih1b = load_w(wih1T_d, 128, H, G, "wih1b")
                whh = {
                    1: (load_w(whh0T_d, 0, 128, G, "whh0a"),
                        load_w(whh0T_d, 128, H, G, "whh0b")),
                    2: (load_w(whh1T_d, 0, 128, G, "whh1a"),
                        load_w(whh1T_d, 128, H, G, "whh1b")),
                }
                fcwa = load_w(fcwT_d, 0, 128, VTOT, "fcwa")
                fcwb = load_w(fcwT_d, 128, H, VTOT, "fcwb")

            # y storage (transposed, f32r): layer-1 out, relu(layer-2 out)
            y1Ta = cp.tile([128, R], f32r, tag="y1Ta", name="y1Ta")
            y1Tb = cp.tile([128, R], f32r, tag="y1Tb", name="y1Tb")
            yrTa = cp.tile([128, R], f32r, tag="yrTa", name="yrTa")
            yrTb = cp.tile([128, R], f32r, tag="yrTb", name="yrTb")

            # h0 gather, one [32, H] base-0 tile per layer:
            # h0b[l][b] = traj20[lab_ix[32l + b]]
            h0b = {}
            for l, li in ((1, 0), (2, 1)):
                ls = cp.tile([B, 1], i32, tag=f"lab{l}", name=f"lab{l}")
                nc.sync.dma_start(ls[:], lab_ix[32 * li:32 * li + 32, :])
                hb = cp.tile([B, H], f32, tag=f"h0b{l}", name=f"h0b{l}")
                nc.gpsimd.indirect_dma_start(
                    out=hb[:], out_offset=None, in_=traj[:],
                    in_offset=bass.IndirectOffsetOnAxis(ap=ls[:, :1], axis=0),
                )
                h0b[l] = hb

            # optional fc bias materialization via ones-matmul broadcast
            fcb_all = None
            if fcb_nz:
                ones_s = cp.tile([1, 128], f32, tag="ones_s", name="ones_s")
                nc.vector.memset(ones_s[:1, :], 1.0)
                ones_r = cp.tile([1, 128], f32r, tag="ones_r", name="ones_r")
                nc.vector.tensor_copy(ones_r[:1, :], ones_s[:1, :])

                with tc.tile_pool(name="bps", bufs=1, space="PSUM") as bps:
                    row_s = cp.tile([1, VTOT], f32, tag="fcb_row_s",
                                    name="fcb_row_s")
                    nc.sync.dma_start(row_s[:1, :], fcb_d[:])
                    row = cp.tile([1, VTOT], f32r, tag="fcb_row",
                                  name="fcb_row")
                    nc.vector.tensor_copy(row[:1, :], row_s[:1, :])
                    fcb_all = cp.tile([128, VTOT], f32, tag="fcb_full",
                                      name="fcb_full")
                    for n0 in range(0, VTOT, 512):
                        n1 = min(n0 + 512, VTOT)
                        pb = bps.tile([128, 512], f32, tag="bps", name="bps")
                        nc.tensor.matmul(pb[:, 0:n1 - n0], ones_r[:1, :],
                                         row[:1, n0:n1], start=True, stop=True)
                        nc.vector.tensor_copy(fcb_all[:, n0:n1],
                                              pb[:, 0:n1 - n0])

            # GRU bias rows (general path): device-side sums/slices of the
            # torch biases, used as K=1 rank-1 matmul contributions.
            brz = {}
            bhn = {}
            bxn = {}
            if gru_bias_nz:
                ones32 = cp.tile([1, 32], f32, tag="ones32", name="ones32")
                nc.vector.memset(ones32[:1, :], 1.0)
                ones32r = cp.tile([1, 32], f32r, tag="ones32r", name="ones32r")
                nc.vector.tensor_copy(ones32r[:1, :], ones32[:1, :])
                for l, gbd in ((1, gb0_d), (2, gb1_d)):
                    # gb = b_ih + b_hh (host passes them separately stacked:
                    # gbd row0 = b_ih, row1 = b_hh)
                    bi = cp.tile([1, G], f32, tag=f"bi{l}", name=f"bi{l}")
                    bh = cp.tile([1, G], f32, tag=f"bh{l}", name=f"bh{l}")
                    nc.sync.dma_start(bi[:1, :], gbd[0:1, :])
                    nc.sync.dma_start(bh[:1, :], gbd[1:2, :])
                    bs = cp.tile([1, 512], f32, tag=f"brzs{l}", name=f"brzs{l}")
                    nc.vector.tensor_tensor(out=bs[:1, :], in0=bi[:1, 0:512],
                                            in1=bh[:1, 0:512], op=ALU.add)
                    br = cp.tile([1, 512], f32r, tag=f"brz{l}", name=f"brz{l}")
                    nc.vector.tensor_copy(br[:1, :], bs[:1, :])
                    bn1 = cp.tile([1, 256], f32r, tag=f"bhn{l}", name=f"bhn{l}")
                    nc.vector.tensor_copy(bn1[:1, :], bh[:1, 512:G])
                    bn2 = cp.tile([1, 256], f32r, tag=f"bxn{l}", name=f"bxn{l}")
                    nc.vector.tensor_copy(bn2[:1, :], bi[:1, 512:G])
                    brz[l], bhn[l], bxn[l] = br, bn1, bn2

            # ---------- embedding + GRU scan (wavefront) ----------
            with (
                tc.tile_pool(name="scan2", bufs=2) as sp2,
                tc.tile_pool(name="scan3", bufs=3) as sp3,
                tc.tile_pool(name="ps_hp", bufs=2, space="PSUM") as psh,
                tc.tile_pool(name="ps_xn", bufs=2, space="PSUM") as psn,
                tc.tile_pool(name="ps_tp", bufs=2, space="PSUM") as pst,
            ):
                def tp_tile():
                    return pst.tile([128, 256], f32, tag="tp", name="tp")

                # h0 transposes -> f32r lhsT tiles
                h0T = {}
                for l in (1, 2):
                    pt = tp_tile()
                    nc.tensor.transpose(pt[:, 0:32], h0b[l][:, 0:128],
                                        idn[0:32, 0:32])
                    nc.tensor.transpose(pt[:, 32:64], h0b[l][:, 128:256],
                                        idn[0:32, 0:32])
                    ta = cp.tile([128, 32], f32r, tag=f"h0T{l}a", name=f"h0T{l}a")
                    tb = cp.tile([128, 32], f32r, tag=f"h0T{l}b", name=f"h0T{l}b")
                    nc.vector.tensor_copy(ta[:], pt[:, 0:32])
                    nc.vector.tensor_copy(tb[:], pt[:, 32:64])
                    h0T[l] = (ta, tb)

                def make_xcatT(c):
                    """Gather embeddings + one-hot for row chunk c; return the
                    transposed lhsT tiles ([128,128] K=0:128, [48,128]
                    K=128:176)."""
                    xc = sp2.tile([128, DIN], f32, tag="xcat", name="xcat")
                    nc.gpsimd.indirect_dma_start(
                        out=xc[:, 0:E], out_offset=None, in_=embed[:],
                        in_offset=bass.IndirectOffsetOnAxis(
                            ap=loc_sb[:, c:c + 1], axis=0),
                    )
                    nc.vector.tensor_scalar(
                        out=xc[:, E:DIN], in0=iota_f[:, 0:TD],
                        scalar1=tim_f[:, c:c + 1], scalar2=None,
                        op0=ALU.is_equal,
                    )
                    pt = tp_tile()
                    nc.tensor.transpose(pt[:, 0:128], xc[:, 0:128],
                                        idn[0:128, 0:128])
                    nc.tensor.transpose(pt[0:48, 128:256], xc[:, 128:DIN],
                                        idn[0:128, 0:128])
                    xa = sp3.tile([128, 128], f32r, tag="xcatTa", name="xcatTa")
                    xb = sp3.tile([48, 128], f32r, tag="xcatTb", name="xcatTb")
                    nc.vector.tensor_copy(xa[:], pt[:, 0:128])
                    nc.vector.tensor_copy(xb[:], pt[0:48, 128:256])
                    return xa, xb

                def gru_step(l, t, xA, xB, wxa, wxb, hprev_b, hTa, hTb):
                    """One GRU step, [32, *] batch-on-partition layout.
                    xA/xB: lhsT slices of the step input ([*,32] f32r);
                    wxa/wxb: matching w_ih rhs tiles. The input projection is
                    fused into the recurrent matmul accumulation groups."""
                    wa, wb = whh[l]
                    hp = psh.tile([32, G], f32, tag="hp", name="hp")
                    # r then z: x-proj + bias + h-proj accumulated in PSUM
                    for n0, n1 in ((0, 256), (256, 512)):
                        nc.tensor.matmul(hp[:, n0:n1], xA, wxa[:, n0:n1],
                                         start=True, stop=False)
                        nc.tensor.matmul(hp[:, n0:n1], xB, wxb[:, n0:n1],
                                         start=False, stop=False)
                        if gru_bias_nz:
                            nc.tensor.matmul(hp[:, n0:n1], ones32r[:1, :],
                                             brz[l][:1, n0:n1],
                                             start=False, stop=False)
                        nc.tensor.matmul(hp[:, n0:n1], hTa, wa[:, n0:n1],
                                         start=False, stop=False)
                        nc.tensor.matmul(hp[:, n0:n1], hTb, wb[:, n0:n1],
                                         start=False, stop=True)
                    # hn = h@whh_n (+ b_hh_n)
                    nc.tensor.matmul(hp[:, 512:G], hTa, wa[:, 512:G],
                                     start=True, stop=False)
                    nc.tensor.matmul(hp[:, 512:G], hTb, wb[:, 512:G],
                                     start=False, stop=not gru_bias_nz)
                    if gru_bias_nz:
                        nc.tensor.matmul(hp[:, 512:G], ones32r[:1, :],
                                         bhn[l][:1, :], start=False, stop=True)
                    # xn = x@wih_n (+ b_ih_n), separate psum
                    xn = psn.tile([32, 256], f32, tag="xn", name="xn")
                    nc.tensor.matmul(xn[:], xA, wxa[:, 512:G],
                                     start=True, stop=False)
                    nc.tensor.matmul(xn[:], xB, wxb[:, 512:G],
                                     start=False, stop=not gru_bias_nz)
                    if gru_bias_nz:
                        nc.tensor.matmul(xn[:], ones32r[:1, :], bxn[l][:1, :],
                                         start=False, stop=True)
                    s = sp2.tile([32, G], f32, tag=f"s{l}", name=f"s{l}")
                    g = sp2.tile([32, G], f32, tag=f"g{l}", name=f"g{l}")
                    nc.scalar.activation(g[:, 0:256], hp[:, 0:256], AF.Sigmoid)
                    nc.scalar.activation(g[:, 256:512], hp[:, 256:512],
                                         AF.Sigmoid)
                    # n gate: tanh(xn + r * hn)
                    nc.vector.tensor_tensor(out=s[:, 512:G], in0=g[:, 0:256],
                                            in1=hp[:, 512:G], op=ALU.mult)
                    nc.vector.tensor_tensor(out=s[:, 512:G], in0=s[:, 512:G],
                                            in1=xn[:], op=ALU.add)
                    nc.scalar.activation(g[:, 512:G], s[:, 512:G], AF.Tanh)
                    # h_new = n*(1-z) + z*h
                    nc.vector.tensor_tensor(out=s[:, 0:256], in0=g[:, 256:512],
                                            in1=hprev_b, op=ALU.mult)  # z*h
                    nc.scalar.activation(s[:, 256:512], g[:, 256:512], AF.Copy,
                                         bias=1.0, scale=-1.0)         # 1-z
                    nc.vector.tensor_tensor(out=g[:, 256:512], in0=g[:, 512:G],
                                            in1=s[:, 256:512], op=ALU.mult)
                    hn = sp2.tile([32, H], f32, tag=f"h{l}", name=f"h{l}")
                    nc.vector.tensor_tensor(out=hn[:], in0=g[:, 256:512],
                                            in1=s[:, 0:256], op=ALU.add)
                    # transpose h_new for the next step's lhsT
                    pt = tp_tile()
                    nc.tensor.transpose(pt[:, 0:32], hn[:, 0:128],
                                        idn[0:32, 0:32])
                    nc.tensor.transpose(pt[:, 32:64], hn[:, 128:256],
                                        idn[0:32, 0:32])
                    if l == 1:
                        nTa = y1Ta[:, 32 * t:32 * t + 32]
                        nTb = y1Tb[:, 32 * t:32 * t + 32]
                        nc.vector.tensor_copy(nTa, pt[:, 0:32])
                        nc.vector.tensor_copy(nTb, pt[:, 32:64])
                    else:
                        ra = sp2.tile([128, 32], f32r, tag="h2Ta", name="h2Ta")
                        rb = sp2.tile([128, 32], f32r, tag="h2Tb", name="h2Tb")
                        nc.vector.tensor_copy(ra[:], pt[:, 0:32])
                        nc.vector.tensor_copy(rb[:], pt[:, 32:64])
                        nc.scalar.activation(yrTa[:, 32 * t:32 * t + 32],
                                             pt[:, 0:32], AF.Relu)
                        nc.scalar.activation(yrTb[:, 32 * t:32 * t + 32],
                                             pt[:, 32:64], AF.Relu)
                        nTa, nTb = ra[:], rb[:]
                    return hn, nTa, nTb

                xcatT = {0: make_xcatT(0), 1: make_xcatT(1)}
                h1b, h2b = h0b[1][:, :], h0b[2][:, :]
                h1Ta, h1Tb = h0T[1][0][:], h0T[1][1][:]
                h2Ta, h2Tb = h0T[2][0][:], h0T[2][1][:]

                def l2_step(t2, h2b, h2Ta, h2Tb):
                    # layer-2 input = layer-1 output at t2 (transposed slices)
                    return gru_step(2, t2,
                                    y1Ta[:, 32 * t2:32 * t2 + 32],
                                    y1Tb[:, 32 * t2:32 * t2 + 32],
                                    wih1a, wih1b, h2b, h2Ta, h2Tb)

                for t in range(T):
                    if t % 4 == 0 and t // 4 + 2 < RC:
                        xcatT[t // 4 + 2] = make_xcatT(t // 4 + 2)
                    xa, xb = xcatT[t // 4]
                    tau = 32 * (t % 4)
                    hn, h1Ta, h1Tb = gru_step(
                        1, t, xa[:, tau:tau + 32], xb[:, tau:tau + 32],
                        wih0a, wih0b, h1b, h1Ta, h1Tb)
                    h1b = hn[:]
                    if t >= 1:
                        hn2, h2Ta, h2Tb = l2_step(t - 1, h2b, h2Ta, h2Tb)
                        h2b = hn2[:]
                hn2, h2Ta, h2Tb = l2_step(T - 1, h2b, h2Ta, h2Tb)

            # ---------- FC phase ----------
            with (
                tc.tile_pool(name="fc1", bufs=1) as fc1,
                tc.tile_pool(name="fc3", bufs=3) as fc3,
                tc.tile_pool(name="fc6", bufs=6) as fc6,
                tc.tile_pool(name="ps_fc", bufs=4, space="PSUM") as fp,
            ):
                accs = fc1.tile([128, RC], f32, tag="accs", name="accs")
                nlz = fc1.tile([128, RC], f32, tag="nlz", name="nlz")

                def fc_mm(pp, c, n0, n1):
                    nc.tensor.matmul(pp[:, 0:n1 - n0],
                                     yrTa[:, 128 * c:128 * c + 128],
                                     fcwa[:, n0:n1], start=True, stop=False)
                    nc.tensor.matmul(pp[:, 0:n1 - n0],
                                     yrTb[:, 128 * c:128 * c + 128],
                                     fcwb[:, n0:n1], start=False, stop=True)

                # pass 1: partial sum(exp(x+b)) over this core's vocab slice
                for c in range(RC):
                    accn = fc3.tile([128, NCH], f32, tag="accn", name="accn")
                    for n in range(NCH):
                        pp = fp.tile([128, 512], f32, tag="fc", name="fc")
                        fc_mm(pp, c, NW * n, NW * (n + 1))
                        es = fc3.tile([128, NW], f32, tag="exps", name="exps")
                        if fcb_nz:
                            nc.vector.tensor_tensor(
                                out=es[:], in0=pp[:, 0:NW],
                                in1=fcb_all[:, NW * n:NW * (n + 1)],
                                op=ALU.add)
                            nc.scalar.activation(es[:], es[:], AF.Exp,
                                                 accum_out=accn[:, n:n + 1])
                        else:
                            nc.scalar.activation(es[:], pp[:, 0:NW], AF.Exp,
                                                 accum_out=accn[:, n:n + 1])
                    nc.vector.tensor_reduce(accs[:, c:c + 1], accn[:],
                                            axis=AX.X, op=ALU.add)

                    # time head: local 48-wide log-softmax
                    pp = fp.tile([128, 512], f32, tag="fc", name="fc")
                    fc_mm(pp, c, VS, VTOT)
                    tme = fc3.tile([128, TD], f32, tag="tme", name="tme")
                    tma = fc3.tile([128, 1], f32, tag="tma", name="tma")
                    if fcb_nz:
                        nc.vector.tensor_tensor(out=tme[:], in0=pp[:, 0:TD],
                                                in1=fcb_all[:, VS:VTOT],
                                                op=ALU.add)
                        nc.scalar.activation(tme[:], tme[:], AF.Exp,
                                             accum_out=tma[:])
                    else:
                        nc.scalar.activation(tme[:], pp[:, 0:TD], AF.Exp,
                                             accum_out=tma[:])
                    tml = fc3.tile([128, 1], f32, tag="tml", name="tml")
                    nc.scalar.activation(tml[:], tma[:], AF.Ln)
                    nc.vector.tensor_scalar_mul(tml[:], tml[:], -1.0)
                    tmo = fc3.tile([128, TD], f32, tag="tmo", name="tmo")
                    if fcb_nz:
                        nc.vector.scalar_tensor_tensor(
                            out=tmo[:], in0=pp[:, 0:TD], scalar=tml[:, :1],
                            in1=fcb_all[:, VS:VTOT], op0=ALU.add, op1=ALU.add)
                    else:
                        nc.vector.tensor_scalar(
                            out=tmo[:], in0=pp[:, 0:TD], scalar1=tml[:, :1],
                            scalar2=None, op0=ALU.add)
                    nc.sync.dma_start(
                        bass.AP(out_tm, 4 * c * TD,
                                [[TD, 4], [T * TD, B], [1, TD]]),
                        tmo[:])

                # AllReduce of the 8 vocab-shard partial sums (6KB)
                cc_in = dp.tile([128, RC], f32, name="cc_in")
                cc_out = dp.tile([128, RC], f32, name="cc_out")
                nc.sync.dma_start(cc_in[:], accs[:])
                nc.gpsimd.collective_compute(
                    "AllReduce", ALU.add,
                    replica_groups=[list(range(NCORES))],
                    ins=[cc_in.opt()], outs=[cc_out.opt()],
                )
                rsums = fc1.tile([128, RC], f32, tag="rsums", name="rsums")
                nc.sync.dma_start(rsums[:], cc_out[:])
                nc.scalar.activation(nlz[:], rsums[:], AF.Ln)
                nc.vector.tensor_scalar_mul(nlz[:], nlz[:], -1.0)

                # pass 2: out = x + b - log(Z)
                for c in range(RC):
                    for n in range(NCH):
                        pp = fp.tile([128, 512], f32, tag="fc", name="fc")
                        fc_mm(pp, c, NW * n, NW * (n + 1))
                        fo = fc6.tile([128, NW], f32, tag="fout", name="fout")
                        if fcb_nz:
                            nc.vector.scalar_tensor_tensor(
                                out=fo[:], in0=pp[:, 0:NW],
                                scalar=nlz[:, c:c + 1],
                                in1=fcb_all[:, NW * n:NW * (n + 1)],
                                op0=ALU.add, op1=ALU.add)
                        elif n % 2 == 0:
                            nc.vector.tensor_scalar(
                                out=fo[:], in0=pp[:, 0:NW],
                                scalar1=nlz[:, c:c + 1],
                                scalar2=None, op0=ALU.add)
                        else:
                            nc.scalar.activation(fo[:], pp[:, 0:NW],
                                                 AF.Identity,
                                                 bias=nlz[:, c:c + 1])
                        nc.sync.dma_start(
                            bass.AP(out_loc, 4 * c * VS + NW * n,
                                    [[VS, 4], [T * VS, B], [1, NW]]),
                            fo[:])

    nc.compile()
    return nc


def _get_nc(gru_bias_nz, fcb_nz):
    key = (gru_bias_nz, fcb_nz)
    if key not in _cache:
        _cache[key] = _build(*key)
    return _cache[key]


def _prep_inputs(locations, times, labels, embed_table, traj_table, fc_w, fc_b,
                 w_ih0, w_hh0, b_ih0, b_hh0, w_ih1, w_hh1, b_ih1, b_hh1):
    f = np.float32
    locations = np.asarray(locations)
    times = np.asarray(times)
    labels = np.asarray(labels)
    # t-major row layout: r = t*32 + b; [128, RC] with [p, c] = row 128c+p
    loc_tm = np.ascontiguousarray(
        locations.T.reshape(RC, 128).T).astype(np.int32)
    tim_tm = np.ascontiguousarray(
        times.T.reshape(RC, 128).T).astype(np.int32)
    # h0 strip-gather indices (torch .view(L, -1, H) semantics):
    # h0[l, b] = traj_table.view(20, 256)[2*labels[16l + b//2] + b%2]
    p = np.arange(2 * B)
    l_, b_ = p // B, p % B
    lab_ix = (2 * labels[(B // 2) * l_ + b_ // 2] + b_ % 2).astype(np.int32)
    lab_ix = np.ascontiguousarray(lab_ix.reshape(2 * B, 1))

    common = dict(
        loc_tm=loc_tm, tim_tm=tim_tm, lab_ix=lab_ix,
        embed=np.ascontiguousarray(embed_table, dtype=f),
        traj=np.ascontiguousarray(
            np.asarray(traj_table, dtype=f).reshape(2 * TRAJ, H)),
        wih0T=np.ascontiguousarray(np.asarray(w_ih0, dtype=f).T),
        wih1T=np.ascontiguousarray(np.asarray(w_ih1, dtype=f).T),
        whh0T=np.ascontiguousarray(np.asarray(w_hh0, dtype=f).T),
        whh1T=np.ascontiguousarray(np.asarray(w_hh1, dtype=f).T),
    )
    b_ih0 = np.asarray(b_ih0, dtype=f)
    b_hh0 = np.asarray(b_hh0, dtype=f)
    b_ih1 = np.asarray(b_ih1, dtype=f)
    b_hh1 = np.asarray(b_hh1, dtype=f)
    gru_bias_nz = bool(np.any(b_ih0) or np.any(b_hh0) or np.any(b_ih1)
                       or np.any(b_hh1))
    if gru_bias_nz:
        common["gb0"] = np.ascontiguousarray(np.stack([b_ih0, b_hh0]))
        common["gb1"] = np.ascontiguousarray(np.stack([b_ih1, b_hh1]))

    fc_w = np.asarray(fc_w, dtype=f)
    fc_b = np.asarray(fc_b, dtype=f)
    fcb_nz = bool(np.any(fc_b))

    in_maps = []
    for c in range(NCORES):
        m = dict(common)
        wslice = np.concatenate([fc_w[c * VS:(c + 1) * VS], fc_w[OUT:]],
                                axis=0)
        m["fcwT"] = np.ascontiguousarray(wslice.T)
        if fcb_nz:
            bslice = np.concatenate([fc_b[c * VS:(c + 1) * VS], fc_b[OUT:]])
            m["fcb"] = np.ascontiguousarray(bslice.reshape(1, VTOT))
        in_maps.append(m)
    return in_maps, gru_bias_nz, fcb_nz


def _run(in_maps, gru_bias_nz, fcb_nz, trace=False):
    from concourse.bass_utils import run_bass_kernel_spmd
    nc = _get_nc(gru_bias_nz, fcb_nz)
    if trace:
        import sys as _sys
        import types as _types
        try:
            from antenv.axon_hooks import get_axon_ntff_profile_hook  # noqa
        except ImportError:
            from trn_agent_boot.trn_boot import _ntff_profile_via_ctypes
            _h = _ntff_profile_via_ctypes('/opt/axon/libaxon_pjrt.so')
            _m = _types.ModuleType('antenv.axon_hooks')
            _m.get_axon_ntff_profile_hook = lambda: _h
            _m.set_axon_ntff_profile_hook = lambda h: None
            _sys.modules['antenv.axon_hooks'] = _m
    return run_bass_kernel_spmd(nc, in_maps, list(range(NCORES)), trace=trace)


def kernel(**inputs):
    in_maps, gru_bias_nz, fcb_nz = _prep_inputs(**inputs)
    res = _run(in_maps, gru_bias_nz, fcb_nz, trace=False)
    loc = np.concatenate([res.results[c]["out_loc"] for c in range(NCORES)],
                         axis=2)
    tm = res.results[0]["out_tm"]
    return loc, tm


# revision 22
# speedup vs baseline: 1.0604x; 1.0604x over previous
"""Trainium2 Bass kernel for nn_MetaDiscreteTimeTrajTypeGRUNet.

Strategy (8 NeuronCores, SPMD):
  - GRU (2 layers, T=48, B=32) replicated on all cores (latency-bound, tiny).
    Wavefront schedule: layer 2 runs 4 steps behind layer 1.
  - FC + log_softmax vocab-parallel: each core owns 5000 output columns
    (+ every core computes the small 48-col time head; core 0's is used).
  - log_softmax without max-subtraction (logits are O(1) by construction):
    pass 1 computes sum(exp(x+b)) per row, one 6KB AllReduce combines the
    8 vocab shards, pass 2 recomputes the matmul (weights stay in SBUF)
    and writes x + b - log(Z) directly.
  - All matmuls in float32r (full PE rate at N>=256).

Row indexing on device is t-major: r = t*32 + b; chunk c = rows 128c..128c+127
(= timesteps 4c..4c+3, all 32 batch elements each). DRAM outputs are written
b-major via strided 3D access patterns.
"""
import numpy as np

B, T, E, TD = 32, 48, 128, 48
H, OUT, TRAJ = 256, 40000, 10
DIN = E + TD          # 176
G = 3 * H             # 768
NCORES = 8
VS = OUT // NCORES    # 5000 vocab columns per core
VTOT = VS + TD        # 5048 fc columns per core (vocab slice + time head)
R = B * T             # 1536
RC = R // 128         # 12 row chunks
NCH = 10              # vocab N-chunks per row chunk
NW = VS // NCH        # 500

_cache = {}


def _build(gru_bias_nz: bool, fcb_nz: bool):
    import concourse.bass as bass
    import concourse.mybir as mybir
    import concourse.tile as tile
    import concourse.bacc as bacc
    from concourse.masks import make_identity

    f32 = mybir.dt.float32
    f32r = mybir.dt.float32r
    i32 = mybir.dt.int32
    AF = mybir.ActivationFunctionType
    ALU = mybir.AluOpType
    AX = mybir.AxisListType

    nc = bacc.Bacc("TRN2", target_bir_lowering=False, debug=False,
                   num_devices=NCORES)

    # ---------------- I/O ----------------
    loc_tm = nc.dram_tensor("loc_tm", [128, RC], i32, kind="ExternalInput")
    tim_tm = nc.dram_tensor("tim_tm", [128, RC], i32, kind="ExternalInput")
    lab_ix = nc.dram_tensor("lab_ix", [2 * B, 1], i32, kind="ExternalInput")
    embed = nc.dram_tensor("embed", [OUT, E], f32, kind="ExternalInput")
    traj = nc.dram_tensor("traj", [2 * TRAJ, H], f32, kind="ExternalInput")
    wih0T_d = nc.dram_tensor("wih0T", [DIN, G], f32, kind="ExternalInput")
    wih1T_d = nc.dram_tensor("wih1T", [H, G], f32, kind="ExternalInput")
    whh0T_d = nc.dram_tensor("whh0T", [H, G], f32, kind="ExternalInput")
    whh1T_d = nc.dram_tensor("whh1T", [H, G], f32, kind="ExternalInput")
    fcwT_d = nc.dram_tensor("fcwT", [H, VTOT], f32, kind="ExternalInput")
    if gru_bias_nz:
        gb0_d = nc.dram_tensor("gb0", [2, G], f32, kind="ExternalInput")
        gb1_d = nc.dram_tensor("gb1", [2, G], f32, kind="ExternalInput")
    if fcb_nz:
        fcb_d = nc.dram_tensor("fcb", [1, VTOT], f32, kind="ExternalInput")

    out_loc = nc.dram_tensor("out_loc", [B, T, VS], f32, kind="ExternalOutput")
    out_tm = nc.dram_tensor("out_tm", [B, T, TD], f32, kind="ExternalOutput")

    with tile.TileContext(nc) as tc:
        with (
            tc.tile_pool(name="const", bufs=1) as cp,
            tc.tile_pool(name="dram", bufs=1, space="DRAM") as dp,
        ):
            # ---------- constants ----------
            idn = cp.tile([128, 128], f32, tag="idn", name="idn")
            make_identity(nc, idn[:])

            iota_i = cp.tile([128, TD], i32, tag="iota_i", name="iota_i")
            nc.gpsimd.iota(iota_i[:], pattern=[[1, TD]], base=0,
                           channel_multiplier=0)
            iota_f = cp.tile([128, TD], f32, tag="iota_f", name="iota_f")
            nc.vector.tensor_copy(iota_f[:], iota_i[:])

            loc_sb = cp.tile([128, RC], i32, tag="loc_sb", name="loc_sb")
            nc.sync.dma_start(loc_sb[:], loc_tm[:])
            tim_i = cp.tile([128, RC], i32, tag="tim_i", name="tim_i")
            nc.sync.dma_start(tim_i[:], tim_tm[:])
            tim_f = cp.tile([128, RC], f32, tag="tim_f", name="tim_f")
            nc.vector.tensor_copy(tim_f[:], tim_i[:])

            # weights -> SBUF as f32r (DMA to a staging tile, then a rounding
            # copy into the f32r tile the matmuls consume)
            with tc.tile_pool(name="stage", bufs=2) as stp:
                def load_w(dram_h, r0, r1, cols, tag):
                    st = stp.tile([r1 - r0, cols], f32, tag="wstage",
                                  name="wstage")
                    nc.sync.dma_start(st[:], dram_h[r0:r1, :])
                    t = cp.tile([r1 - r0, cols], f32r, tag=tag, name=tag)
                    nc.vector.tensor_copy(t[:], st[:])
                    return t

                wih0a = load_w(wih0T_d, 0, 128, G, "wih0a")
                wih0b = load_w(wih0T_d, 128, DIN, G, "wih0b")
                wih1a = load_w(wih1T_d, 0, 128, G, "wih1a")
                wih1b = load_w(wih1T_d, 128, H, G, "wih1b")
                whh = {
                    1: (load_w(whh0T_d, 0, 128, G, "whh0a"),
                        load_w(whh0T_d, 128, H, G, "whh0b")),
                    2: (load_w(whh1T_d, 0, 128, G, "whh1a"),
                        load_w(whh1T_d, 128, H, G, "whh1b")),
                }
                fcwa = load_w(fcwT_d, 0, 128, VTOT, "fcwa")
                fcwb = load_w(fcwT_d, 128, H, VTOT, "fcwb")

            # y storage (transposed, f32r): layer-1 out, relu(layer-2 out)
            y1Ta = cp.tile([128, R], f32r, tag="y1Ta", name="y1Ta")
            y1Tb = cp.tile([128, R], f32r, tag="y1Tb", name="y1Tb")
            yrTa = cp.tile([128, R], f32r, tag="yrTa", name="yrTa")
            yrTb = cp.tile([128, R], f32r, tag="yrTb", name="yrTb")

            # h0 gather, one [32, H] base-0 tile per layer:
            # h0b[l][b] = traj20[lab_ix[32l + b]]
            h0b = {}
            for l, li in ((1, 0), (2, 1)):
                ls = cp.tile([B, 1], i32, tag=f"lab{l}", name=f"lab{l}")
                nc.sync.dma_start(ls[:], lab_ix[32 * li:32 * li + 32, :])
                hb = cp.tile([B, H], f32, tag=f"h0b{l}", name=f"h0b{l}")
                nc.gpsimd.indirect_dma_start(
                    out=hb[:], out_offset=None, in_=traj[:],
                    in_offset=bass.IndirectOffsetOnAxis(ap=ls[:, :1], axis=0),
                )
                h0b[l] = hb

            # optional fc bias materialization via ones-matmul broadcast
            fcb_all = None
            if fcb_nz:
                ones_s = cp.tile([1, 128], f32, tag="ones_s", name="ones_s")
                nc.vector.memset(ones_s[:1, :], 1.0)
                ones_r = cp.tile([1, 128], f32r, tag="ones_r", name="ones_r")
                nc.vector.tensor_copy(ones_r[:1, :], ones_s[:1, :])

                with tc.tile_pool(name="bps", bufs=1, space="PSUM") as bps:
                    row_s = cp.tile([1, VTOT], f32, tag="fcb_row_s",
                                    name="fcb_row_s")
                    nc.sync.dma_start(row_s[:1, :], fcb_d[:])
                    row = cp.tile([1, VTOT], f32r, tag="fcb_row",
                                  name="fcb_row")
                    nc.vector.tensor_copy(row[:1, :], row_s[:1, :])
                    fcb_all = cp.tile([128, VTOT], f32, tag="fcb_full",
                                      name="fcb_full")
                    for n0 in range(0, VTOT, 512):
                        n1 = min(n0 + 512, VTOT)
                        pb = bps.tile([128, 512], f32, tag="bps", name="bps")
                        nc.tensor.matmul(pb[:, 0:n1 - n0], ones_r[:1, :],
                                         row[:1, n0:n1], start=True, stop=True)
                        nc.vector.tensor_copy(fcb_all[:, n0:n1],
                                              pb[:, 0:n1 - n0])

            # GRU bias rows (general path): device-side sums/slices of the
            # torch biases, used as K=1 rank-1 matmul contributions.
            brz = {}
            bhn = {}
            bxn = {}
            if gru_bias_nz:
                ones32 = cp.tile([1, 32], f32, tag="ones32", name="ones32")
                nc.vector.memset(ones32[:1, :], 1.0)
                ones32r = cp.tile([1, 32], f32r, tag="ones32r", name="ones32r")
                nc.vector.tensor_copy(ones32r[:1, :], ones32[:1, :])
                for l, gbd in ((1, gb0_d), (2, gb1_d)):
                    # gb = b_ih + b_hh (host passes them separately stacked:
                    # gbd row0 = b_ih, row1 = b_hh)
                    bi = cp.tile([1, G], f32, tag=f"bi{l}", name=f"bi{l}")
                    bh = cp.tile([1, G], f32, tag=f"bh{l}", name=f"bh{l}")
                    nc.sync.dma_start(bi[:1, :], gbd[0:1, :])
                    nc.sync.dma_start(bh[:1, :], gbd[1:2, :])
                    bs = cp.tile([1, 512], f32, tag=f"brzs{l}", name=f"brzs{l}")
                    nc.vector.tensor_tensor(out=bs[:1, :], in0=bi[:1, 0:512],
                                            in1=bh[:1, 0:512], op=ALU.add)
                    br = cp.tile([1, 512], f32r, tag=f"brz{l}", name=f"brz{l}")
                    nc.vector.tensor_copy(br[:1, :], bs[:1, :])
                    bn1 = cp.tile([1, 256], f32r, tag=f"bhn{l}", name=f"bhn{l}")
                    nc.vector.tensor_copy(bn1[:1, :], bh[:1, 512:G])
                    bn2 = cp.tile([1, 256], f32r, tag=f"bxn{l}", name=f"bxn{l}")
                    nc.vector.tensor_copy(bn2[:1, :], bi[:1, 512:G])
                    brz[l], bhn[l], bxn[l] = br, bn1, bn2

            # ---------- embedding + GRU scan (wavefront) ----------
            with (
                tc.tile_pool(name="scan2", bufs=2) as sp2,
                tc.tile_pool(name="scan3", bufs=3) as sp3,
                tc.tile_pool(name="ps_hp", bufs=2, space="PSUM") as psh,
                tc.tile_pool(name="ps_xn", bufs=2, space="PSUM") as psn,
                tc.tile_pool(name="ps_tp", bufs=2, space="PSUM") as pst,
            ):
                def tp_tile():
                    return pst.tile([128, 256], f32, tag="tp", name="tp")

                # h0 transposes -> f32r lhsT tiles
                h0T = {}
                for l in (1, 2):
                    pt = tp_tile()
                    nc.tensor.transpose(pt[:, 0:32], h0b[l][:, 0:128],
                                        idn[0:32, 0:32])
                    nc.tensor.transpose(pt[:, 32:64], h0b[l][:, 128:256],
                                        idn[0:32, 0:32])
                    ta = cp.tile([128, 32], f32r, tag=f"h0T{l}a", name=f"h0T{l}a")
                    tb = cp.tile([128, 32], f32r, tag=f"h0T{l}b", name=f"h0T{l}b")
                    nc.vector.tensor_copy(ta[:], pt[:, 0:32])
                    nc.vector.tensor_copy(tb[:], pt[:, 32:64])
                    h0T[l] = (ta, tb)

                def make_xcatT(c):
                    """Gather embeddings + one-hot for row chunk c; return the
                    transposed lhsT tiles ([128,128] K=0:128, [48,128]
                    K=128:176)."""
                    xc = sp2.tile([128, DIN], f32, tag="xcat", name="xcat")
                    nc.gpsimd.indirect_dma_start(
                        out=xc[:, 0:E], out_offset=None, in_=embed[:],
                        in_offset=bass.IndirectOffsetOnAxis(
                            ap=loc_sb[:, c:c + 1], axis=0),
                    )
                    nc.vector.tensor_scalar(
                        out=xc[:, E:DIN], in0=iota_f[:, 0:TD],
                        scalar1=tim_f[:, c:c + 1], scalar2=None,
                        op0=ALU.is_equal,
                    )
                    pt = tp_tile()
                    nc.tensor.transpose(pt[:, 0:128], xc[:, 0:128],
                                        idn[0:128, 0:128])
                    nc.tensor.transpose(pt[0:48, 128:256], xc[:, 128:DIN],
                                        idn[0:128, 0:128])
                    xa = sp3.tile([128, 128], f32r, tag="xcatTa", name="xcatTa")
                    xb = sp3.tile([48, 128], f32r, tag="xcatTb", name="xcatTb")
                    nc.vector.tensor_copy(xa[:], pt[:, 0:128])
                    nc.vector.tensor_copy(xb[:], pt[0:48, 128:256])
                    return xa, xb

                def gru_step(l, t, xA, xB, wxa, wxb, hprev_b, hTa, hTb):
                    """One GRU step, [32, *] batch-on-partition layout.
                    xA/xB: lhsT slices of the step input ([*,32] f32r);
                    wxa/wxb: matching w_ih rhs tiles. The input projection is
                    fused into the recurrent matmul accumulation groups."""
                    wa, wb = whh[l]
                    hp = psh.tile([32, G], f32, tag="hp", name="hp")
                    # r then z: x-proj + bias + h-proj accumulated in PSUM
                    for n0, n1 in ((0, 256), (256, 512)):
                        nc.tensor.matmul(hp[:, n0:n1], xA, wxa[:, n0:n1],
                                         start=True, stop=False)
                        nc.tensor.matmul(hp[:, n0:n1], xB, wxb[:, n0:n1],
                                         start=False, stop=False)
                        if gru_bias_nz:
                            nc.tensor.matmul(hp[:, n0:n1], ones32r[:1, :],
                                             brz[l][:1, n0:n1],
                                             start=False, stop=False)
                        nc.tensor.matmul(hp[:, n0:n1], hTa, wa[:, n0:n1],
                                         start=False, stop=False)
                        nc.tensor.matmul(hp[:, n0:n1], hTb, wb[:, n0:n1],
                                         start=False, stop=True)
                    # hn = h@whh_n (+ b_hh_n)
                    nc.tensor.matmul(hp[:, 512:G], hTa, wa[:, 512:G],
                                     start=True, stop=False)
                    nc.tensor.matmul(hp[:, 512:G], hTb, wb[:, 512:G],
                                     start=False, stop=not gru_bias_nz)
                    if gru_bias_nz:
                        nc.tensor.matmul(hp[:, 512:G], ones32r[:1, :],
                                         bhn[l][:1, :], start=False, stop=True)
                    # xn = x@wih_n (+ b_ih_n), separate psum
                    xn = psn.tile([32, 256], f32, tag="xn", name="xn")
                    nc.tensor.matmul(xn[:], xA, wxa[:, 512:G],
                                     start=True, stop=False)
                    nc.tensor.matmul(xn[:], xB, wxb[:, 512:G],
                                     start=False, stop=not gru_bias_nz)
                    if gru_bias_nz:
                        nc.tensor.matmul(xn[:], ones32r[:1, :], bxn[l][:1, :],
                                         start=False, stop=True)
                    s = sp2.tile([32, G], f32, tag=f"s{l}", name=f"s{l}")
                    g = sp2.tile([32, G], f32, tag=f"g{l}", name=f"g{l}")
                    nc.scalar.activation(g[:, 0:256], hp[:, 0:256], AF.Sigmoid)
                    nc.scalar.activation(g[:, 256:512], hp[:, 256:512],
                                         AF.Sigmoid)
                    # n gate: tanh(xn + r * hn)
                    nc.vector.tensor_tensor(out=s[:, 512:G], in0=g[:, 0:256],
                                            in1=hp[:, 512:G], op=ALU.mult)
                    nc.vector.tensor_tensor(out=s[:, 512:G], in0=s[:, 512:G],
                                            in1=xn[:], op=ALU.add)
                    nc.scalar.activation(g[:, 512:G], s[:, 512:G], AF.Tanh)
                    # h_new = n*(1-z) + z*h
                    nc.vector.tensor_tensor(out=s[:, 0:256], in0=g[:, 256:512],
                                            in1=hprev_b, op=ALU.mult)  # z*h
                    nc.scalar.activation(s[:, 256:512], g[:, 256:512], AF.Copy,
                                         bias=1.0, scale=-1.0)         # 1-z
                    nc.vector.tensor_tensor(out=g[:, 256:512], in0=g[:, 512:G],
                                            in1=s[:, 256:512], op=ALU.mult)
                    hn = sp2.tile([32, H], f32, tag=f"h{l}", name=f"h{l}")
                    nc.vector.tensor_tensor(out=hn[:], in0=g[:, 256:512],
                                            in1=s[:, 0:256], op=ALU.add)
                    # transpose h_new for the next step's lhsT
                    pt = tp_tile()
                    nc.tensor.transpose(pt[:, 0:32], hn[:, 0:128],
                                        idn[0:32, 0:32])
                    nc.tensor.transpose(pt[:, 32:64], hn[:, 128:256],
                                        idn[0:32, 0:32])
                    if l == 1:
                        nTa = y1Ta[:, 32 * t:32 * t + 32]
                        nTb = y1Tb[:, 32 * t:32 * t + 32]
                        nc.vector.tensor_copy(nTa, pt[:, 0:32])
                        nc.vector.tensor_copy(nTb, pt[:, 32:64])
                    else:
                        ra = sp2.tile([128, 32], f32r, tag="h2Ta", name="h2Ta")
                        rb = sp2.tile([128, 32], f32r, tag="h2Tb", name="h2Tb")
                        nc.vector.tensor_copy(ra[:], pt[:, 0:32])
                        nc.vector.tensor_copy(rb[:], pt[:, 32:64])
                        nc.scalar.activation(yrTa[:, 32 * t:32 * t + 32],
                                             pt[:, 0:32], AF.Relu)
                        nc.scalar.activation(yrTb[:, 32 * t:32 * t + 32],
                                             pt[:, 32:64], AF.Relu)
                        nTa, nTb = ra[:], rb[:]
                    return hn, nTa, nTb

                xcatT = {0: make_xcatT(0), 1: make_xcatT(1)}
                h1b, h2b = h0b[1][:, :], h0b[2][:, :]
                h1Ta, h1Tb = h0T[1][0][:], h0T[1][1][:]
                h2Ta, h2Tb = h0T[2][0][:], h0T[2][1][:]

                def l2_step(t2, h2b, h2Ta, h2Tb):
                    # layer-2 input = layer-1 output at t2 (transposed slices)
                    return gru_step(2, t2,
                                    y1Ta[:, 32 * t2:32 * t2 + 32],
                                    y1Tb[:, 32 * t2:32 * t2 + 32],
                                    wih1a, wih1b, h2b, h2Ta, h2Tb)

                for t in range(T):
                    if t % 4 == 0 and t // 4 + 2 < RC:
                        xcatT[t // 4 + 2] = make_xcatT(t // 4 + 2)
                    xa, xb = xcatT[t // 4]
                    tau = 32 * (t % 4)
                    hn, h1Ta, h1Tb = gru_step(
                        1, t, xa[:, tau:tau + 32], xb[:, tau:tau + 32],
                        wih0a, wih0b, h1b, h1Ta, h1Tb)
                    h1b = hn[:]
                    if t >= 1:
                        hn2, h2Ta, h2Tb = l2_step(t - 1, h2b, h2Ta, h2Tb)
                        h2b = hn2[:]
                hn2, h2Ta, h2Tb = l2_step(T - 1, h2b, h2Ta, h2Tb)

            # ---------- FC phase ----------
            with (
                tc.tile_pool(name="fc1", bufs=1) as fc1,
                tc.tile_pool(name="fc3", bufs=3) as fc3,
                tc.tile_pool(name="fc6", bufs=6) as fc6,
                tc.tile_pool(name="ps_fc", bufs=4, space="PSUM") as fp,
            ):
                accs = fc1.tile([128, RC], f32, tag="accs", name="accs")
                nlz = fc1.tile([128, RC], f32, tag="nlz", name="nlz")
                tm_all = fc1.tile([128, RC * TD], f32, tag="tm_all",
                                  name="tm_all")

                def fc_mm(pp, c, n0, n1):
                    nc.tensor.matmul(pp[:, 0:n1 - n0],
                                     yrTa[:, 128 * c:128 * c + 128],
                                     fcwa[:, n0:n1], start=True, stop=False)
                    nc.tensor.matmul(pp[:, 0:n1 - n0],
                                     yrTb[:, 128 * c:128 * c + 128],
                                     fcwb[:, n0:n1], start=False, stop=True)

                # pass 1: partial sum(exp(x+b)) over this core's vocab slice
                for c in range(RC):
                    accn = fc3.tile([128, NCH], f32, tag="accn", name="accn")
                    for n in range(NCH):
                        pp = fp.tile([128, 512], f32, tag="fc", name="fc")
                        fc_mm(pp, c, NW * n, NW * (n + 1))
                        es = fc3.tile([128, NW], f32, tag="exps", name="exps")
                        if fcb_nz:
                            nc.vector.tensor_tensor(
                                out=es[:], in0=pp[:, 0:NW],
                                in1=fcb_all[:, NW * n:NW * (n + 1)],
                                op=ALU.add)
                            nc.scalar.activation(es[:], es[:], AF.Exp,
                                                 accum_out=accn[:, n:n + 1])
                        else:
                            nc.scalar.activation(es[:], pp[:, 0:NW], AF.Exp,
                                                 accum_out=accn[:, n:n + 1])
                    nc.vector.tensor_reduce(accs[:, c:c + 1], accn[:],
                                            axis=AX.X, op=ALU.add)

                    # time head: local 48-wide log-softmax
                    pp = fp.tile([128, 512], f32, tag="fc", name="fc")
                    fc_mm(pp, c, VS, VTOT)
                    tme = fc3.tile([128, TD], f32, tag="tme", name="tme")
                    tma = fc3.tile([128, 1], f32, tag="tma", name="tma")
                    if fcb_nz:
                        nc.vector.tensor_tensor(out=tme[:], in0=pp[:, 0:TD],
                                                in1=fcb_all[:, VS:VTOT],
                                                op=ALU.add)
                        nc.scalar.activation(tme[:], tme[:], AF.Exp,
                                             accum_out=tma[:])
                    else:
                        nc.scalar.activation(tme[:], pp[:, 0:TD], AF.Exp,
                                             accum_out=tma[:])
                    tml = fc3.tile([128, 1], f32, tag="tml", name="tml")
                    nc.scalar.activation(tml[:], tma[:], AF.Ln)
                    nc.vector.tensor_scalar_mul(tml[:], tml[:], -1.0)
                    tmo = tm_all[:, TD * c:TD * (c + 1)]
                    if fcb_nz:
                        nc.vector.scalar_tensor_tensor(
                            out=tmo, in0=pp[:, 0:TD], scalar=tml[:, :1],
                            in1=fcb_all[:, VS:VTOT], op0=ALU.add, op1=ALU.add)
                    else:
                        nc.vector.tensor_scalar(
                            out=tmo, in0=pp[:, 0:TD], scalar1=tml[:, :1],
                            scalar2=None, op0=ALU.add)

                # single DMA for the whole time-head output
                nc.sync.dma_start(
                    bass.AP(out_tm, 0,
                            [[TD, 4], [T * TD, B], [4 * TD, RC], [1, TD]]),
                    tm_all[:])

                # AllReduce of the 8 vocab-shard partial sums (6KB)
                cc_in = dp.tile([128, RC], f32, name="cc_in")
                cc_out = dp.tile([128, RC], f32, name="cc_out")
                nc.sync.dma_start(cc_in[:], accs[:])
                nc.gpsimd.collective_compute(
                    "AllReduce", ALU.add,
                    replica_groups=[list(range(NCORES))],
                    ins=[cc_in.opt()], outs=[cc_out.opt()],
                )
                rsums = fc1.tile([128, RC], f32, tag="rsums", name="rsums")
                nc.sync.dma_start(rsums[:], cc_out[:])
                nc.scalar.activation(nlz[:], rsums[:], AF.Ln)
                nc.vector.tensor_scalar_mul(nlz[:], nlz[:], -1.0)

                # pass 2: out = x + b - log(Z); one batched 2.5MB DMA per
                # row chunk (128 partition-lines of 20KB)
                for c in range(RC):
                    fo = fc6.tile([128, VS], f32, tag="fout", name="fout",
                                  bufs=2)
                    for n in range(NCH):
                        pp = fp.tile([128, 512], f32, tag="fc", name="fc")
                        fc_mm(pp, c, NW * n, NW * (n + 1))
                        fs_ = fo[:, NW * n:NW * (n + 1)]
                        if fcb_nz:
                            nc.vector.scalar_tensor_tensor(
                                out=fs_, in0=pp[:, 0:NW],
                                scalar=nlz[:, c:c + 1],
                                in1=fcb_all[:, NW * n:NW * (n + 1)],
                                op0=ALU.add, op1=ALU.add)
                        elif n % 2 == 0:
                            nc.vector.tensor_scalar(
                                out=fs_, in0=pp[:, 0:NW],
                                scalar1=nlz[:, c:c + 1],
                                scalar2=None, op0=ALU.add)
                        else:
                            nc.scalar.activation(fs_, pp[:, 0:NW],
                                                 AF.Identity,
                                                 bias=nlz[:, c:c + 1])
                    nc.sync.dma_start(
                        bass.AP(out_loc, 4 * c * VS,
                                [[VS, 4], [T * VS, B], [1, VS]]),
                        fo[:])

    nc.compile()
    return nc


def _get_nc(gru_bias_nz, fcb_nz):
    key = (gru_bias_nz, fcb_nz)
    if key not in _cache:
        _cache[key] = _build(*key)
    return _cache[key]


def _prep_inputs(locations, times, labels, embed_table, traj_table, fc_w, fc_b,
                 w_ih0, w_hh0, b_ih0, b_hh0, w_ih1, w_hh1, b_ih1, b_hh1):
    f = np.float32
    locations = np.asarray(locations)
    times = np.asarray(times)
    labels = np.asarray(labels)
    # t-major row layout: r = t*32 + b; [128, RC] with [p, c] = row 128c+p
    loc_tm = np.ascontiguousarray(
        locations.T.reshape(RC, 128).T).astype(np.int32)
    tim_tm = np.ascontiguousarray(
        times.T.reshape(RC, 128).T).astype(np.int32)
    # h0 strip-gather indices (torch .view(L, -1, H) semantics):
    # h0[l, b] = traj_table.view(20, 256)[2*labels[16l + b//2] + b%2]
    p = np.arange(2 * B)
    l_, b_ = p // B, p % B
    lab_ix = (2 * labels[(B // 2) * l_ + b_ // 2] + b_ % 2).astype(np.int32)
    lab_ix = np.ascontiguousarray(lab_ix.reshape(2 * B, 1))

    common = dict(
        loc_tm=loc_tm, tim_tm=tim_tm, lab_ix=lab_ix,
        embed=np.ascontiguousarray(embed_table, dtype=f),
        traj=np.ascontiguousarray(
            np.asarray(traj_table, dtype=f).reshape(2 * TRAJ, H)),
        wih0T=np.ascontiguousarray(np.asarray(w_ih0, dtype=f).T),
        wih1T=np.ascontiguousarray(np.asarray(w_ih1, dtype=f).T),
        whh0T=np.ascontiguousarray(np.asarray(w_hh0, dtype=f).T),
        whh1T=np.ascontiguousarray(np.asarray(w_hh1, dtype=f).T),
    )
    b_ih0 = np.asarray(b_ih0, dtype=f)
    b_hh0 = np.asarray(b_hh0, dtype=f)
    b_ih1 = np.asarray(b_ih1, dtype=f)
    b_hh1 = np.asarray(b_hh1, dtype=f)
    gru_bias_nz = bool(np.any(b_ih0) or np.any(b_hh0) or np.any(b_ih1)
                       or np.any(b_hh1))
    if gru_bias_nz:
        common["gb0"] = np.ascontiguousarray(np.stack([b_ih0, b_hh0]))
        common["gb1"] = np.ascontiguousarray(np.stack([b_ih1, b_hh1]))

    fc_w = np.asarray(fc_w, dtype=f)
    fc_b = np.asarray(fc_b, dtype=f)
    fcb_nz = bool(np.any(fc_b))

    in_maps = []
    for c in range(NCORES):
        m = dict(common)
        wslice = np.concatenate([fc_w[c * VS:(c + 1) * VS], fc_w[OUT:]],
                                axis=0)
        m["fcwT"] = np.ascontiguousarray(wslice.T)
        if fcb_nz:
            bslice = np.concatenate([fc_b[c * VS:(c + 1) * VS], fc_b[OUT:]])
            m["fcb"] = np.ascontiguousarray(bslice.reshape(1, VTOT))
        in_maps.append(m)
    return in_maps, gru_bias_nz, fcb_nz


def _run(in_maps, gru_bias_nz, fcb_nz, trace=False):
    from concourse.bass_utils import run_bass_kernel_spmd
    nc = _get_nc(gru_bias_nz, fcb_nz)
    if trace:
        import sys as _sys
        import types as _types
        try:
            from antenv.axon_hooks import get_axon_ntff_profile_hook  # noqa
        except ImportError:
            from trn_agent_boot.trn_boot import _ntff_profile_via_ctypes
            _h = _ntff_profile_via_ctypes('/opt/axon/libaxon_pjrt.so')
            _m = _types.ModuleType('antenv.axon_hooks')
            _m.get_axon_ntff_profile_hook = lambda: _h
            _m.set_axon_ntff_profile_hook = lambda h: None
            _sys.modules['antenv.axon_hooks'] = _m
    return run_bass_kernel_spmd(nc, in_maps, list(range(NCORES)), trace=trace)


def kernel(**inputs):
    in_maps, gru_bias_nz, fcb_nz = _prep_inputs(**inputs)
    res = _run(in_maps, gru_bias_nz, fcb_nz, trace=False)
    loc = np.concatenate([res.results[c]["out_loc"] for c in range(NCORES)],
                         axis=2)
    tm = res.results[0]["out_tm"]
    return loc, tm


# revision 25
# speedup vs baseline: 1.1415x; 1.0766x over previous
"""Trainium2 Bass kernel for nn_MetaDiscreteTimeTrajTypeGRUNet.

Strategy (8 NeuronCores, SPMD):
  - GRU (2 layers, T=48, B=32) replicated on all cores (latency-bound, tiny).
    Wavefront schedule: layer 2 runs 1 step behind layer 1; the input
    projections (x@w_ih) are fused into the recurrent PSUM accumulation
    groups, so there is no separate xp precompute.
  - FC + log_softmax vocab-parallel: each core owns 5000 output columns
    (+ every core computes the small 48-col time head; core 0's is used).
  - log_softmax without max-subtraction (logits are O(1) by construction):
    pass 1 (interleaved into the scan as row chunks finish) computes
    sum(exp(x+b)) per row; three staged 2KB AllReduces combine the 8 vocab
    shards (the first two complete during the scan); pass 2 recomputes the
    matmul (weights stay in SBUF) and writes x + b - log(Z) with one
    batched 2.5MB DMA per row chunk.
  - All matmuls in float32r (full PE rate at N>=256).

Row indexing on device is t-major: r = t*32 + b; chunk c = rows 128c..128c+127
(= timesteps 4c..4c+3, all 32 batch elements each). DRAM outputs are written
b-major via strided 3D access patterns.
"""
import numpy as np

B, T, E, TD = 32, 48, 128, 48
H, OUT, TRAJ = 256, 40000, 10
DIN = E + TD          # 176
G = 3 * H             # 768
NCORES = 8
VS = OUT // NCORES    # 5000 vocab columns per core
VTOT = VS + TD        # 5048 fc columns per core (vocab slice + time head)
R = B * T             # 1536
RC = R // 12          # noqa: E501  (unused alias guard)
RC = R // 128         # 12 row chunks
NCH = 10              # vocab N-chunks per row chunk
NW = VS // NCH        # 500
NGRP = 3              # allreduce groups (4 row chunks each)

_cache = {}


def _build(gru_bias_nz: bool, fcb_nz: bool):
    import concourse.bass as bass
    import concourse.mybir as mybir
    import concourse.tile as tile
    import concourse.bacc as bacc
    from concourse.masks import make_identity

    f32 = mybir.dt.float32
    f32r = mybir.dt.float32r
    i32 = mybir.dt.int32
    AF = mybir.ActivationFunctionType
    ALU = mybir.AluOpType
    AX = mybir.AxisListType

    nc = bacc.Bacc("TRN2", target_bir_lowering=False, debug=False,
                   num_devices=NCORES)

    # ---------------- I/O ----------------
    loc_tm = nc.dram_tensor("loc_tm", [128, RC], i32, kind="ExternalInput")
    tim_tm = nc.dram_tensor("tim_tm", [128, RC], i32, kind="ExternalInput")
    lab_ix = nc.dram_tensor("lab_ix", [2 * B, 1], i32, kind="ExternalInput")
    embed = nc.dram_tensor("embed", [OUT, E], f32, kind="ExternalInput")
    traj = nc.dram_tensor("traj", [2 * TRAJ, H], f32, kind="ExternalInput")
    wih0T_d = nc.dram_tensor("wih0T", [DIN, G], f32, kind="ExternalInput")
    wih1T_d = nc.dram_tensor("wih1T", [H, G], f32, kind="ExternalInput")
    whh0T_d = nc.dram_tensor("whh0T", [H, G], f32, kind="ExternalInput")
    whh1T_d = nc.dram_tensor("whh1T", [H, G], f32, kind="ExternalInput")
    fcwT_d = nc.dram_tensor("fcwT", [H, VTOT], f32, kind="ExternalInput")
    if gru_bias_nz:
        gb0_d = nc.dram_tensor("gb0", [2, G], f32, kind="ExternalInput")
        gb1_d = nc.dram_tensor("gb1", [2, G], f32, kind="ExternalInput")
    if fcb_nz:
        fcb_d = nc.dram_tensor("fcb", [1, VTOT], f32, kind="ExternalInput")

    out_loc = nc.dram_tensor("out_loc", [B, T, VS], f32, kind="ExternalOutput")
    out_tm = nc.dram_tensor("out_tm", [B, T, TD], f32, kind="ExternalOutput")

    with tile.TileContext(nc) as tc:
        with (
            tc.tile_pool(name="const", bufs=1) as cp,
            tc.tile_pool(name="dram", bufs=1, space="DRAM") as dp,
        ):
            # ---------- constants ----------
            idn = cp.tile([128, 128], f32, tag="idn", name="idn")
            make_identity(nc, idn[:])

            iota_i = cp.tile([128, TD], i32, tag="iota_i", name="iota_i")
            nc.gpsimd.iota(iota_i[:], pattern=[[1, TD]], base=0,
                           channel_multiplier=0)
            iota_f = cp.tile([128, TD], f32, tag="iota_f", name="iota_f")
            nc.vector.tensor_copy(iota_f[:], iota_i[:])

            loc_sb = cp.tile([128, RC], i32, tag="loc_sb", name="loc_sb")
            nc.sync.dma_start(loc_sb[:], loc_tm[:])
            tim_i = cp.tile([128, RC], i32, tag="tim_i", name="tim_i")
            nc.sync.dma_start(tim_i[:], tim_tm[:])
            tim_f = cp.tile([128, RC], f32, tag="tim_f", name="tim_f")
            nc.vector.tensor_copy(tim_f[:], tim_i[:])

            # weights -> SBUF as f32r via staging + rounding copy
            with tc.tile_pool(name="stage", bufs=2) as stp:
                def load_w(dram_h, r0, r1, cols, tag):
                    st = stp.tile([r1 - r0, cols], f32, tag="wstage",
                                  name="wstage")
                    nc.sync.dma_start(st[:], dram_h[r0:r1, :])
                    t = cp.tile([r1 - r0, cols], f32r, tag=tag, name=tag)
                    nc.vector.tensor_copy(t[:], st[:])
                    return t

                wih0a = load_w(wih0T_d, 0, 128, G, "wih0a")
                wih0b = load_w(wih0T_d, 128, DIN, G, "wih0b")
                wih1a = load_w(wih1T_d, 0, 128, G, "wih1a")
                wih1b = load_w(wih1T_d, 128, H, G, "wih1b")
                whh = {
                    1: (load_w(whh0T_d, 0, 128, G, "whh0a"),
                        load_w(whh0T_d, 128, H, G, "whh0b")),
                    2: (load_w(whh1T_d, 0, 128, G, "whh1a"),
                        load_w(whh1T_d, 128, H, G, "whh1b")),
                }
                fcwa = load_w(fcwT_d, 0, 128, VTOT, "fcwa")
                fcwb = load_w(fcwT_d, 128, H, VTOT, "fcwb")

            # transposed y storage, two K-halves side by side:
            # [:, 0:R] = K rows 0:128, [:, R:2R] = K rows 128:256
            y1T = cp.tile([128, 2 * R], f32r, tag="y1T", name="y1T")
            yrT = cp.tile([128, 2 * R], f32r, tag="yrT", name="yrT")

            # h0 gather, one [32, H] base-0 tile per layer
            h0b = {}
            for l, li in ((1, 0), (2, 1)):
                ls = cp.tile([B, 1], i32, tag=f"lab{l}", name=f"lab{l}")
                nc.sync.dma_start(ls[:], lab_ix[32 * li:32 * li + 32, :])
                hb = cp.tile([B, H], f32, tag=f"h0b{l}", name=f"h0b{l}")
                nc.gpsimd.indirect_dma_start(
                    out=hb[:], out_offset=None, in_=traj[:],
                    in_offset=bass.IndirectOffsetOnAxis(ap=ls[:, :1], axis=0),
                )
                h0b[l] = hb

            # optional fc bias materialization via ones-matmul broadcast
            fcb_all = None
            if fcb_nz:
                ones_s = cp.tile([1, 128], f32, tag="ones_s", name="ones_s")
                nc.vector.memset(ones_s[:1, :], 1.0)
                ones_r = cp.tile([1, 128], f32r, tag="ones_r", name="ones_r")
                nc.vector.tensor_copy(ones_r[:1, :], ones_s[:1, :])
                with tc.tile_pool(name="bps", bufs=1, space="PSUM") as bps:
                    row_s = cp.tile([1, VTOT], f32, tag="fcb_row_s",
                                    name="fcb_row_s")
                    nc.sync.dma_start(row_s[:1, :], fcb_d[:])
                    row = cp.tile([1, VTOT], f32r, tag="fcb_row",
                                  name="fcb_row")
                    nc.vector.tensor_copy(row[:1, :], row_s[:1, :])
                    fcb_all = cp.tile([128, VTOT], f32, tag="fcb_full",
                                      name="fcb_full")
                    for n0 in range(0, VTOT, 512):
                        n1 = min(n0 + 512, VTOT)
                        pb = bps.tile([128, 512], f32, tag="bps", name="bps")
                        nc.tensor.matmul(pb[:, 0:n1 - n0], ones_r[:1, :],
                                         row[:1, n0:n1], start=True, stop=True)
                        nc.vector.tensor_copy(fcb_all[:, n0:n1],
                                              pb[:, 0:n1 - n0])

            # GRU bias rows (general path) as K=1 rank-1 matmul contributions
            brz = {}
            bhn = {}
            bxn = {}
            if gru_bias_nz:
                ones32 = cp.tile([1, 32], f32, tag="ones32", name="ones32")
                nc.vector.memset(ones32[:1, :], 1.0)
                ones32r = cp.tile([1, 32], f32r, tag="ones32r", name="ones32r")
                nc.vector.tensor_copy(ones32r[:1, :], ones32[:1, :])
                for l, gbd in ((1, gb0_d), (2, gb1_d)):
                    bi = cp.tile([1, G], f32, tag=f"bi{l}", name=f"bi{l}")
                    bh = cp.tile([1, G], f32, tag=f"bh{l}", name=f"bh{l}")
                    nc.sync.dma_start(bi[:1, :], gbd[0:1, :])
                    nc.sync.dma_start(bh[:1, :], gbd[1:2, :])
                    bs = cp.tile([1, 512], f32, tag=f"brzs{l}",
                                 name=f"brzs{l}")
                    nc.vector.tensor_tensor(out=bs[:1, :], in0=bi[:1, 0:512],
                                            in1=bh[:1, 0:512], op=ALU.add)
                    br = cp.tile([1, 512], f32r, tag=f"brz{l}", name=f"brz{l}")
                    nc.vector.tensor_copy(br[:1, :], bs[:1, :])
                    bn1 = cp.tile([1, 256], f32r, tag=f"bhn{l}",
                                  name=f"bhn{l}")
                    nc.vector.tensor_copy(bn1[:1, :], bh[:1, 512:G])
                    bn2 = cp.tile([1, 256], f32r, tag=f"bxn{l}",
                                  name=f"bxn{l}")
                    nc.vector.tensor_copy(bn2[:1, :], bi[:1, 512:G])
                    brz[l], bhn[l], bxn[l] = br, bn1, bn2

            # ---------- scan + interleaved FC pass 1 ----------
            with (
                tc.tile_pool(name="scan2", bufs=2) as sp2,
                tc.tile_pool(name="scan3", bufs=3) as sp3,
                tc.tile_pool(name="fcp1", bufs=1) as fc1,
                tc.tile_pool(name="fcp3", bufs=3) as fc3,
                tc.tile_pool(name="ps_hp", bufs=2, space="PSUM") as psh,
                tc.tile_pool(name="ps_xn", bufs=1, space="PSUM") as psn,
                tc.tile_pool(name="ps_tp", bufs=1, space="PSUM") as pst,
                tc.tile_pool(name="ps_fc", bufs=2, space="PSUM") as fp,
            ):
                accs = fc1.tile([128, RC], f32, tag="accs", name="accs")
                nlz = fc1.tile([128, RC], f32, tag="nlz", name="nlz")
                rsums = fc1.tile([128, RC], f32, tag="rsums", name="rsums")
                tm_all = fc1.tile([128, RC * TD], f32, tag="tm_all",
                                  name="tm_all")
                cc_in = [dp.tile([128, 4], f32, name=f"cc_in{g}")
                         for g in range(NGRP)]
                cc_out = [dp.tile([128, 4], f32, name=f"cc_out{g}")
                          for g in range(NGRP)]

                def tp_tile():
                    return pst.tile([128, 256], f32, tag="tp", name="tp")

                # h0 transposes -> f32r lhsT tiles
                h0T = {}
                for l in (1, 2):
                    pt = tp_tile()
                    nc.tensor.transpose(pt[:, 0:32], h0b[l][:, 0:128],
                                        idn[0:32, 0:32])
                    nc.tensor.transpose(pt[:, 32:64], h0b[l][:, 128:256],
                                        idn[0:32, 0:32])
                    ht = cp.tile([128, 64], f32r, tag=f"h0T{l}",
                                 name=f"h0T{l}")
                    nc.vector.tensor_copy(ht[:], pt[:, 0:64])
                    h0T[l] = ht

                def make_xcatT(c):
                    xc = sp2.tile([128, DIN], f32, tag="xcat", name="xcat")
                    nc.gpsimd.indirect_dma_start(
                        out=xc[:, 0:E], out_offset=None, in_=embed[:],
                        in_offset=bass.IndirectOffsetOnAxis(
                            ap=loc_sb[:, c:c + 1], axis=0),
                    )
                    nc.vector.tensor_scalar(
                        out=xc[:, E:DIN], in0=iota_f[:, 0:TD],
                        scalar1=tim_f[:, c:c + 1], scalar2=None,
                        op0=ALU.is_equal,
                    )
                    pt = tp_tile()
                    nc.tensor.transpose(pt[:, 0:128], xc[:, 0:128],
                                        idn[0:128, 0:128])
                    nc.tensor.transpose(pt[0:48, 128:256], xc[:, 128:DIN],
                                        idn[0:128, 0:128])
                    xa = sp3.tile([128, 128], f32r, tag="xcatTa",
                                  name="xcatTa")
                    xb = sp3.tile([48, 128], f32r, tag="xcatTb", name="xcatTb")
                    nc.vector.tensor_copy(xa[:], pt[:, 0:128])
                    nc.vector.tensor_copy(xb[:], pt[0:48, 128:256])
                    return xa, xb

                def gru_step(l, t, xA, xB, wxa, wxb, hprev_b, hTa, hTb):
                    """One GRU step, [32, *] batch-on-partition layout."""
                    wa, wb = whh[l]
                    hp = psh.tile([32, G], f32, tag="hp", name="hp")
                    # r+z: one N=512 accumulation group (x-proj + bias + h)
                    nc.tensor.matmul(hp[:, 0:512], xA, wxa[:, 0:512],
                                     start=True, stop=False)
                    nc.tensor.matmul(hp[:, 0:512], xB, wxb[:, 0:512],
                                     start=False, stop=False)
                    if gru_bias_nz:
                        nc.tensor.matmul(hp[:, 0:512], ones32r[:1, :],
                                         brz[l][:1, :], start=False,
                                         stop=False)
                    nc.tensor.matmul(hp[:, 0:512], hTa, wa[:, 0:512],
                                     start=False, stop=False)
                    nc.tensor.matmul(hp[:, 0:512], hTb, wb[:, 0:512],
                                     start=False, stop=True)
                    # hn
                    nc.tensor.matmul(hp[:, 512:G], hTa, wa[:, 512:G],
                                     start=True, stop=False)
                    nc.tensor.matmul(hp[:, 512:G], hTb, wb[:, 512:G],
                                     start=False, stop=not gru_bias_nz)
                    if gru_bias_nz:
                        nc.tensor.matmul(hp[:, 512:G], ones32r[:1, :],
                                         bhn[l][:1, :], start=False, stop=True)
                    # xn
                    xn = psn.tile([32, 256], f32, tag="xn", name="xn")
                    nc.tensor.matmul(xn[:], xA, wxa[:, 512:G],
                                     start=True, stop=False)
                    nc.tensor.matmul(xn[:], xB, wxb[:, 512:G],
                                     start=False, stop=not gru_bias_nz)
                    if gru_bias_nz:
                        nc.tensor.matmul(xn[:], ones32r[:1, :], bxn[l][:1, :],
                                         start=False, stop=True)
                    s = sp2.tile([32, G], f32, tag=f"s{l}", name=f"s{l}")
                    g = sp2.tile([32, G], f32, tag=f"g{l}", name=f"g{l}")
                    # r,z in one activation straight from PSUM
                    nc.scalar.activation(g[:, 0:512], hp[:, 0:512], AF.Sigmoid)
                    # n = tanh(xn + r*hn)
                    nc.vector.tensor_tensor(out=s[:, 512:G], in0=g[:, 0:256],
                                            in1=hp[:, 512:G], op=ALU.mult)
                    nc.vector.tensor_tensor(out=s[:, 512:G], in0=s[:, 512:G],
                                            in1=xn[:], op=ALU.add)
                    nc.scalar.activation(g[:, 512:G], s[:, 512:G], AF.Tanh)
                    # h_new = n + z*(h_prev - n)
                    nc.vector.tensor_tensor(out=s[:, 0:256], in0=hprev_b,
                                            in1=g[:, 512:G], op=ALU.subtract)
                    nc.vector.tensor_tensor(out=s[:, 256:512],
                                            in0=g[:, 256:512],
                                            in1=s[:, 0:256], op=ALU.mult)
                    hn = sp2.tile([32, H], f32, tag=f"h{l}", name=f"h{l}")
                    nc.vector.tensor_tensor(out=hn[:], in0=g[:, 512:G],
                                            in1=s[:, 256:512], op=ALU.add)
                    # transpose h_new for the next step's lhsT (+ storage)
                    pt = tp_tile()
                    nc.tensor.transpose(pt[:, 0:32], hn[:, 0:128],
                                        idn[0:32, 0:32])
                    nc.tensor.transpose(pt[:, 32:64], hn[:, 128:256],
                                        idn[0:32, 0:32])
                    pt2 = pt[:, 0:64].rearrange("p (k r) -> p k r", k=2)
                    if l == 1:
                        dst = y1T.rearrange("p (k r) -> p k r",
                                            k=2)[:, :, 32 * t:32 * t + 32]
                        nc.vector.tensor_copy(dst, pt2)
                        nTa = y1T[:, 32 * t:32 * t + 32]
                        nTb = y1T[:, R + 32 * t:R + 32 * t + 32]
                    else:
                        ht = sp2.tile([128, 64], f32r, tag="h2T", name="h2T")
                        nc.vector.tensor_copy(ht[:], pt[:, 0:64])
                        rdst = yrT.rearrange("p (k r) -> p k r",
                                             k=2)[:, :, 32 * t:32 * t + 32]
                        nc.scalar.activation(rdst, pt2, AF.Relu)
                        nTa, nTb = ht[:, 0:32], ht[:, 32:64]
                    return hn, nTa, nTb

                # FC pass-1 quarter-chunks: compute sum(exp(x+b)) per row
                NPARTS = ((0, 1, 2), (3, 4, 5), (6, 7, 8), (9, -1))
                accn_tiles = {}

                def fc_mm(pp, c, n0, n1):
                    nc.tensor.matmul(pp[:, 0:n1 - n0],
                                     yrT[:, 128 * c:128 * c + 128],
                                     fcwa[:, n0:n1], start=True, stop=False)
                    nc.tensor.matmul(pp[:, 0:n1 - n0],
                                     yrT[:, R + 128 * c:R + 128 * c + 128],
                                     fcwb[:, n0:n1], start=False, stop=True)

                def pass1_part(c, part):
                    if part == 0:
                        accn_tiles[c] = fc3.tile([128, NCH], f32, tag="accn",
                                                 name="accn")
                    accn = accn_tiles[c]
                    for n in NPARTS[part]:
                        if n >= 0:
                            pp = fp.tile([128, 512], f32, tag="fc", name="fc")
                            fc_mm(pp, c, NW * n, NW * (n + 1))
                            es = fc3.tile([128, NW], f32, tag="exps",
                                          name="exps")
                            if fcb_nz:
                                nc.vector.tensor_tensor(
                                    out=es[:], in0=pp[:, 0:NW],
                                    in1=fcb_all[:, NW * n:NW * (n + 1)],
                                    op=ALU.add)
                                nc.scalar.activation(
                                    es[:], es[:], AF.Exp,
                                    accum_out=accn[:, n:n + 1])
                            else:
                                nc.scalar.activation(
                                    es[:], pp[:, 0:NW], AF.Exp,
                                    accum_out=accn[:, n:n + 1])
                        else:
                            # time head: local 48-wide log-softmax
                            pp = fp.tile([128, 512], f32, tag="fc", name="fc")
                            fc_mm(pp, c, VS, VTOT)
                            tme = fc3.tile([128, TD], f32, tag="tme",
                                           name="tme")
                            tma = fc3.tile([128, 1], f32, tag="tma",
                                           name="tma")
                            if fcb_nz:
                                nc.vector.tensor_tensor(
                                    out=tme[:], in0=pp[:, 0:TD],
                                    in1=fcb_all[:, VS:VTOT], op=ALU.add)
                                nc.scalar.activation(tme[:], tme[:], AF.Exp,
                                                     accum_out=tma[:])
                            else:
                                nc.scalar.activation(tme[:], pp[:, 0:TD],
                                                     AF.Exp, accum_out=tma[:])
                            tml = fc3.tile([128, 1], f32, tag="tml",
                                           name="tml")
                            nc.scalar.activation(tml[:], tma[:], AF.Ln)
                            nc.vector.tensor_scalar_mul(tml[:], tml[:], -1.0)
                            tmo = tm_all[:, TD * c:TD * (c + 1)]
                            if fcb_nz:
                                nc.vector.scalar_tensor_tensor(
                                    out=tmo, in0=pp[:, 0:TD],
                                    scalar=tml[:, :1],
                                    in1=fcb_all[:, VS:VTOT],
                                    op0=ALU.add, op1=ALU.add)
                            else:
                                nc.vector.tensor_scalar(
                                    out=tmo, in0=pp[:, 0:TD],
                                    scalar1=tml[:, :1], scalar2=None,
                                    op0=ALU.add)
                            nc.vector.tensor_reduce(
                                accs[:, c:c + 1], accn[:], axis=AX.X,
                                op=ALU.add)

                def launch_ar(g):
                    nc.sync.dma_start(cc_in[g][:], accs[:, 4 * g:4 * g + 4])
                    nc.gpsimd.collective_compute(
                        "AllReduce", ALU.add,
                        replica_groups=[list(range(NCORES))],
                        ins=[cc_in[g].opt()], outs=[cc_out[g].opt()],
                    )

                # ---------- the scan loop ----------
                xcatT = {0: make_xcatT(0), 1: make_xcatT(1)}
                h1b, h2b = h0b[1][:, :], h0b[2][:, :]
                h1Ta, h1Tb = h0T[1][:, 0:32], h0T[1][:, 32:64]
                h2Ta, h2Tb = h0T[2][:, 0:32], h0T[2][:, 32:64]

                def l2_step(t2, h2b, h2Ta, h2Tb):
                    return gru_step(2, t2,
                                    y1T[:, 32 * t2:32 * t2 + 32],
                                    y1T[:, R + 32 * t2:R + 32 * t2 + 32],
                                    wih1a, wih1b, h2b, h2Ta, h2Tb)

                for t in range(T):
                    if t % 4 == 0 and t // 4 + 2 < RC:
                        xcatT[t // 4 + 2] = make_xcatT(t // 4 + 2)
                    xa, xb = xcatT[t // 4]
                    tau = 32 * (t % 4)
                    hn, h1Ta, h1Tb = gru_step(
                        1, t, xa[:, tau:tau + 32], xb[:, tau:tau + 32],
                        wih0a, wih0b, h1b, h1Ta, h1Tb)
                    h1b = hn[:]
                    if t >= 1:
                        hn2, h2Ta, h2Tb = l2_step(t - 1, h2b, h2Ta, h2Tb)
                        h2b = hn2[:]
                    # interleave FC pass 1 for finished row chunks:
                    # chunk c is done after layer-2 step 4c+3 (wave 4c+4)
                    if t >= 4:
                        c, part = (t - 4) // 4, (t - 4) % 4
                        if c < RC - 1:
                            pass1_part(c, part)
                    if t == 20:
                        launch_ar(0)
                    if t == 36:
                        launch_ar(1)
                hn2, h2Ta, h2Tb = l2_step(T - 1, h2b, h2Ta, h2Tb)
                for part in range(4):
                    pass1_part(RC - 1, part)
                launch_ar(2)

                # collect allreduced sums -> -log(Z); group 2's collect is
                # deferred until after pass-2 chunk 7 so its wait does not
                # block the ACT/DVE queues for chunks 0..7
                def collect_group(g):
                    nc.sync.dma_start(rsums[:, 4 * g:4 * g + 4],
                                      cc_out[g][:])
                    nc.scalar.activation(nlz[:, 4 * g:4 * g + 4],
                                         rsums[:, 4 * g:4 * g + 4], AF.Ln)
                    nc.vector.tensor_scalar_mul(nlz[:, 4 * g:4 * g + 4],
                                                nlz[:, 4 * g:4 * g + 4], -1.0)

                collect_group(0)
                collect_group(1)

                # ---------- FC pass 2 ----------
                with tc.tile_pool(name="fout_p", bufs=2) as fop:
                    for c in range(RC):
                        if c == 8:
                            collect_group(2)
                        fo = fop.tile([128, VS], f32, tag="fout", name="fout")
                        for n in range(NCH):
                            pp = fp.tile([128, 512], f32, tag="fc", name="fc")
                            fc_mm(pp, c, NW * n, NW * (n + 1))
                            fs_ = fo[:, NW * n:NW * (n + 1)]
                            if fcb_nz:
                                nc.vector.scalar_tensor_tensor(
                                    out=fs_, in0=pp[:, 0:NW],
                                    scalar=nlz[:, c:c + 1],
                                    in1=fcb_all[:, NW * n:NW * (n + 1)],
                                    op0=ALU.add, op1=ALU.add)
                            elif n % 2 == 0:
                                nc.vector.tensor_scalar(
                                    out=fs_, in0=pp[:, 0:NW],
                                    scalar1=nlz[:, c:c + 1],
                                    scalar2=None, op0=ALU.add)
                            else:
                                nc.scalar.activation(fs_, pp[:, 0:NW],
                                                     AF.Identity,
                                                     bias=nlz[:, c:c + 1])
                        nc.sync.dma_start(
                            bass.AP(out_loc, 4 * c * VS,
                                    [[VS, 4], [T * VS, B], [1, VS]]),
                            fo[:])

                # single DMA for the whole time-head output
                nc.sync.dma_start(
                    bass.AP(out_tm, 0,
                            [[TD, 4], [T * TD, B], [4 * TD, RC], [1, TD]]),
                    tm_all[:])

    nc.compile()
    return nc


def _get_nc(gru_bias_nz, fcb_nz):
    key = (gru_bias_nz, fcb_nz)
    if key not in _cache:
        _cache[key] = _build(*key)
    return _cache[key]


def _prep_inputs(locations, times, labels, embed_table, traj_table, fc_w, fc_b,
                 w_ih0, w_hh0, b_ih0, b_hh0, w_ih1, w_hh1, b_ih1, b_hh1):
    f = np.float32
    locations = np.asarray(locations)
    times = np.asarray(times)
    labels = np.asarray(labels)
    # t-major row layout: r = t*32 + b; [128, RC] with [p, c] = row 128c+p
    loc_tm = np.ascontiguousarray(
        locations.T.reshape(RC, 128).T).astype(np.int32)
    tim_tm = np.ascontiguousarray(
        times.T.reshape(RC, 128).T).astype(np.int32)
    # h0 strip-gather indices (torch .view(L, -1, H) semantics):
    # h0[l, b] = traj_table.view(20, 256)[2*labels[16l + b//2] + b%2]
    p = np.arange(2 * B)
    l_, b_ = p // B, p % B
    lab_ix = (2 * labels[(B // 2) * l_ + b_ // 2] + b_ % 2).astype(np.int32)
    lab_ix = np.ascontiguousarray(lab_ix.reshape(2 * B, 1))

    common = dict(
        loc_tm=loc_tm, tim_tm=tim_tm, lab_ix=lab_ix,
        embed=np.ascontiguousarray(embed_table, dtype=f),
        traj=np.ascontiguousarray(
            np.asarray(traj_table, dtype=f).reshape(2 * TRAJ, H)),
        wih0T=np.ascontiguousarray(np.asarray(w_ih0, dtype=f).T),
        wih1T=np.ascontiguousarray(np.asarray(w_ih1, dtype=f).T),
        whh0T=np.ascontiguousarray(np.asarray(w_hh0, dtype=f).T),
        whh1T=np.ascontiguousarray(np.asarray(w_hh1, dtype=f).T),
    )
    b_ih0 = np.asarray(b_ih0, dtype=f)
    b_hh0 = np.asarray(b_hh0, dtype=f)
    b_ih1 = np.asarray(b_ih1, dtype=f)
    b_hh1 = np.asarray(b_hh1, dtype=f)
    gru_bias_nz = bool(np.any(b_ih0) or np.any(b_hh0) or np.any(b_ih1)
                       or np.any(b_hh1))
    if gru_bias_nz:
        common["gb0"] = np.ascontiguousarray(np.stack([b_ih0, b_hh0]))
        common["gb1"] = np.ascontiguousarray(np.stack([b_ih1, b_hh1]))

    fc_w = np.asarray(fc_w, dtype=f)
    fc_b = np.asarray(fc_b, dtype=f)
    fcb_nz = bool(np.any(fc_b))

    in_maps = []
    for c in range(NCORES):
        m = dict(common)
        wslice = np.concatenate([fc_w[c * VS:(c + 1) * VS], fc_w[OUT:]],
                                axis=0)
        m["fcwT"] = np.ascontiguousarray(wslice.T)
        if fcb_nz:
            bslice = np.concatenate([fc_b[c * VS:(c + 1) * VS], fc_b[OUT:]])
            m["fcb"] = np.ascontiguousarray(bslice.reshape(1, VTOT))
        in_maps.append(m)
    return in_maps, gru_bias_nz, fcb_nz


def _run(in_maps, gru_bias_nz, fcb_nz, trace=False):
    from concourse.bass_utils import run_bass_kernel_spmd
    nc = _get_nc(gru_bias_nz, fcb_nz)
    if trace:
        import sys as _sys
        import types as _types
        try:
            from antenv.axon_hooks import get_axon_ntff_profile_hook  # noqa
        except ImportError:
            from trn_agent_boot.trn_boot import _ntff_profile_via_ctypes
            _h = _ntff_profile_via_ctypes('/opt/axon/libaxon_pjrt.so')
            _m = _types.ModuleType('antenv.axon_hooks')
            _m.get_axon_ntff_profile_hook = lambda: _h
            _m.set_axon_ntff_profile_hook = lambda h: None
            _sys.modules['antenv.axon_hooks'] = _m
    return run_bass_kernel_spmd(nc, in_maps, list(range(NCORES)), trace=trace)


def kernel(**inputs):
    in_maps, gru_bias_nz, fcb_nz = _prep_inputs(**inputs)
    res = _run(in_maps, gru_bias_nz, fcb_nz, trace=False)
    loc = np.concatenate([res.results[c]["out_loc"] for c in range(NCORES)],
                         axis=2)
    tm = res.results[0]["out_tm"]
    return loc, tm


# revision 31
# speedup vs baseline: 1.4800x; 1.2965x over previous
"""Trainium2 Bass kernel for nn_MetaDiscreteTimeTrajTypeGRUNet.

Strategy (8 NeuronCores, SPMD):
  - GRU (2 layers, T=48, B=32) replicated on all cores (latency-bound, tiny).
    Wavefront schedule: layer 2 runs 1 step behind layer 1; the input
    projections (x@w_ih) are fused into the recurrent PSUM accumulation
    groups, so there is no separate xp precompute.
  - FC + log_softmax vocab-parallel: each core owns 5000 output columns
    (+ every core computes the small 48-col time head; core 0's is used).
  - log_softmax without max-subtraction (logits are O(1) by construction):
    pass 1 (interleaved into the scan as row chunks finish) computes
    sum(exp(x+b)) per row; three staged 2KB AllReduces combine the 8 vocab
    shards (the first two complete during the scan); pass 2 recomputes the
    matmul (weights stay in SBUF) and writes x + b - log(Z) with one
    batched 2.5MB DMA per row chunk.
  - All matmuls in float32r (full PE rate at N>=256).

Row indexing on device is t-major: r = t*32 + b; chunk c = rows 128c..128c+127
(= timesteps 4c..4c+3, all 32 batch elements each). DRAM outputs are written
b-major via strided 3D access patterns.
"""
import numpy as np

B, T, E, TD = 32, 48, 128, 48
H, OUT, TRAJ = 256, 40000, 10
DIN = E + TD          # 176
G = 3 * H             # 768
NCORES = 8
VS = OUT // NCORES    # 5000 vocab columns per core
VTOT = VS + TD        # 5048 fc columns per core (vocab slice + time head)
R = B * T             # 1536
RC = R // 12          # noqa: E501  (unused alias guard)
RC = R // 128         # 12 row chunks
NCH = 10              # vocab N-chunks per row chunk
NW = VS // NCH        # 500
NGRP = 3              # allreduce groups (4 row chunks each)

_cache = {}


def _build(gru_bias_nz: bool, fcb_nz: bool):
    import concourse.bass as bass
    import concourse.mybir as mybir
    import concourse.tile as tile
    import concourse.bacc as bacc
    from concourse.masks import make_identity

    f32 = mybir.dt.float32
    f32r = mybir.dt.float32r
    i32 = mybir.dt.int32
    AF = mybir.ActivationFunctionType
    ALU = mybir.AluOpType
    AX = mybir.AxisListType

    nc = bacc.Bacc("TRN2", target_bir_lowering=False, debug=False,
                   num_devices=NCORES)

    # ---------------- I/O ----------------
    loc_tm = nc.dram_tensor("loc_tm", [128, RC], i32, kind="ExternalInput")
    tim_tm = nc.dram_tensor("tim_tm", [128, RC], i32, kind="ExternalInput")
    lab_ix = nc.dram_tensor("lab_ix", [2 * B, 1], i32, kind="ExternalInput")
    embed = nc.dram_tensor("embed", [OUT, E], f32, kind="ExternalInput")
    traj = nc.dram_tensor("traj", [2 * TRAJ, H], f32, kind="ExternalInput")
    wih0T_d = nc.dram_tensor("wih0T", [DIN, G], f32, kind="ExternalInput")
    wih1T_d = nc.dram_tensor("wih1T", [H, G], f32, kind="ExternalInput")
    whh0T_d = nc.dram_tensor("whh0T", [H, G], f32, kind="ExternalInput")
    whh1T_d = nc.dram_tensor("whh1T", [H, G], f32, kind="ExternalInput")
    fcwT_d = nc.dram_tensor("fcwT", [H, VTOT], f32, kind="ExternalInput")
    if gru_bias_nz:
        gb0_d = nc.dram_tensor("gb0", [2, G], f32, kind="ExternalInput")
        gb1_d = nc.dram_tensor("gb1", [2, G], f32, kind="ExternalInput")
    if fcb_nz:
        fcb_d = nc.dram_tensor("fcb", [1, VTOT], f32, kind="ExternalInput")

    # outputs are written t-major (row r = t*32+b, contiguous 2.5MB blocks
    # per row chunk — ~3.4x the DMA bandwidth of b-major strided writes);
    # the host reindexes to [B, T, *]
    out_loc = nc.dram_tensor("out_loc", [R, VS], f32, kind="ExternalOutput")
    out_tm = nc.dram_tensor("out_tm", [R, TD], f32, kind="ExternalOutput")

    with tile.TileContext(nc) as tc:
        with (
            tc.tile_pool(name="const", bufs=1) as cp,
            tc.tile_pool(name="dram", bufs=1, space="DRAM") as dp,
        ):
            # ---------- constants ----------
            idn = cp.tile([128, 128], f32, tag="idn", name="idn")
            make_identity(nc, idn[:])

            iota_i = cp.tile([128, TD], i32, tag="iota_i", name="iota_i")
            nc.gpsimd.iota(iota_i[:], pattern=[[1, TD]], base=0,
                           channel_multiplier=0)
            iota_f = cp.tile([128, TD], f32, tag="iota_f", name="iota_f")
            nc.vector.tensor_copy(iota_f[:], iota_i[:])

            loc_sb = cp.tile([128, RC], i32, tag="loc_sb", name="loc_sb")
            nc.sync.dma_start(loc_sb[:], loc_tm[:])
            tim_i = cp.tile([128, RC], i32, tag="tim_i", name="tim_i")
            nc.sync.dma_start(tim_i[:], tim_tm[:])
            tim_f = cp.tile([128, RC], f32, tag="tim_f", name="tim_f")
            nc.vector.tensor_copy(tim_f[:], tim_i[:])

            # weights -> SBUF as f32r via staging + rounding copy
            with tc.tile_pool(name="stage", bufs=2) as stp:
                def load_w(dram_h, r0, r1, cols, tag):
                    st = stp.tile([r1 - r0, cols], f32, tag="wstage",
                                  name="wstage")
                    nc.sync.dma_start(st[:], dram_h[r0:r1, :])
                    t = cp.tile([r1 - r0, cols], f32r, tag=tag, name=tag)
                    nc.vector.tensor_copy(t[:], st[:])
                    return t

                wih0a = load_w(wih0T_d, 0, 128, G, "wih0a")
                wih0b = load_w(wih0T_d, 128, DIN, G, "wih0b")
                wih1a = load_w(wih1T_d, 0, 128, G, "wih1a")
                wih1b = load_w(wih1T_d, 128, H, G, "wih1b")
                whh = {
                    1: (load_w(whh0T_d, 0, 128, G, "whh0a"),
                        load_w(whh0T_d, 128, H, G, "whh0b")),
                    2: (load_w(whh1T_d, 0, 128, G, "whh1a"),
                        load_w(whh1T_d, 128, H, G, "whh1b")),
                }
                fcwa = load_w(fcwT_d, 0, 128, VTOT, "fcwa")
                fcwb = load_w(fcwT_d, 128, H, VTOT, "fcwb")

            # transposed y storage, two K-halves side by side:
            # [:, 0:R] = K rows 0:128, [:, R:2R] = K rows 128:256
            y1T = cp.tile([128, 2 * R], f32r, tag="y1T", name="y1T")
            yrT = cp.tile([128, 2 * R], f32r, tag="yrT", name="yrT")

            # h0 gather, one [32, H] base-0 tile per layer
            h0b = {}
            for l, li in ((1, 0), (2, 1)):
                ls = cp.tile([B, 1], i32, tag=f"lab{l}", name=f"lab{l}")
                nc.sync.dma_start(ls[:], lab_ix[32 * li:32 * li + 32, :])
                hb = cp.tile([B, H], f32, tag=f"h0b{l}", name=f"h0b{l}")
                nc.gpsimd.indirect_dma_start(
                    out=hb[:], out_offset=None, in_=traj[:],
                    in_offset=bass.IndirectOffsetOnAxis(ap=ls[:, :1], axis=0),
                )
                h0b[l] = hb

            # optional fc bias materialization via ones-matmul broadcast
            fcb_all = None
            if fcb_nz:
                ones_s = cp.tile([1, 128], f32, tag="ones_s", name="ones_s")
                nc.vector.memset(ones_s[:1, :], 1.0)
                ones_r = cp.tile([1, 128], f32r, tag="ones_r", name="ones_r")
                nc.vector.tensor_copy(ones_r[:1, :], ones_s[:1, :])
                with tc.tile_pool(name="bps", bufs=1, space="PSUM") as bps:
                    row_s = cp.tile([1, VTOT], f32, tag="fcb_row_s",
                                    name="fcb_row_s")
                    nc.sync.dma_start(row_s[:1, :], fcb_d[:])
                    row = cp.tile([1, VTOT], f32r, tag="fcb_row",
                                  name="fcb_row")
                    nc.vector.tensor_copy(row[:1, :], row_s[:1, :])
                    fcb_all = cp.tile([128, VTOT], f32, tag="fcb_full",
                                      name="fcb_full")
                    for n0 in range(0, VTOT, 512):
                        n1 = min(n0 + 512, VTOT)
                        pb = bps.tile([128, 512], f32, tag="bps", name="bps")
                        nc.tensor.matmul(pb[:, 0:n1 - n0], ones_r[:1, :],
                                         row[:1, n0:n1], start=True, stop=True)
                        nc.vector.tensor_copy(fcb_all[:, n0:n1],
                                              pb[:, 0:n1 - n0])

            # GRU bias rows (general path) as K=1 rank-1 matmul contributions
            brz = {}
            bhn = {}
            bxn = {}
            if gru_bias_nz:
                ones32 = cp.tile([1, 32], f32, tag="ones32", name="ones32")
                nc.vector.memset(ones32[:1, :], 1.0)
                ones32r = cp.tile([1, 32], f32r, tag="ones32r", name="ones32r")
                nc.vector.tensor_copy(ones32r[:1, :], ones32[:1, :])
                for l, gbd in ((1, gb0_d), (2, gb1_d)):
                    bi = cp.tile([1, G], f32, tag=f"bi{l}", name=f"bi{l}")
                    bh = cp.tile([1, G], f32, tag=f"bh{l}", name=f"bh{l}")
                    nc.sync.dma_start(bi[:1, :], gbd[0:1, :])
                    nc.sync.dma_start(bh[:1, :], gbd[1:2, :])
                    bs = cp.tile([1, 512], f32, tag=f"brzs{l}",
                                 name=f"brzs{l}")
                    nc.vector.tensor_tensor(out=bs[:1, :], in0=bi[:1, 0:512],
                                            in1=bh[:1, 0:512], op=ALU.add)
                    br = cp.tile([1, 512], f32r, tag=f"brz{l}", name=f"brz{l}")
                    nc.vector.tensor_copy(br[:1, :], bs[:1, :])
                    bn1 = cp.tile([1, 256], f32r, tag=f"bhn{l}",
                                  name=f"bhn{l}")
                    nc.vector.tensor_copy(bn1[:1, :], bh[:1, 512:G])
                    bn2 = cp.tile([1, 256], f32r, tag=f"bxn{l}",
                                  name=f"bxn{l}")
                    nc.vector.tensor_copy(bn2[:1, :], bi[:1, 512:G])
                    brz[l], bhn[l], bxn[l] = br, bn1, bn2

            # ---------- scan + interleaved FC pass 1 ----------
            with (
                tc.tile_pool(name="scan2", bufs=2) as sp2,
                tc.tile_pool(name="scan3", bufs=3) as sp3,
                tc.tile_pool(name="fcp1", bufs=1) as fc1,
                tc.tile_pool(name="fcp3", bufs=3) as fc3,
                tc.tile_pool(name="ps_hp", bufs=2, space="PSUM") as psh,
                tc.tile_pool(name="ps_xn", bufs=1, space="PSUM") as psn,
                tc.tile_pool(name="ps_tp", bufs=1, space="PSUM") as pst,
                tc.tile_pool(name="ps_fc", bufs=2, space="PSUM") as fp,
            ):
                accs = fc1.tile([128, RC], f32, tag="accs", name="accs")
                nlz = fc1.tile([128, RC], f32, tag="nlz", name="nlz")
                rsums = fc1.tile([128, RC], f32, tag="rsums", name="rsums")
                tm_all = fc1.tile([128, RC * TD], f32, tag="tm_all",
                                  name="tm_all")
                cc_in = [dp.tile([128, 4], f32, name=f"cc_in{g}")
                         for g in range(NGRP)]
                cc_out = [dp.tile([128, 4], f32, name=f"cc_out{g}")
                          for g in range(NGRP)]

                def tp_tile():
                    return pst.tile([128, 256], f32, tag="tp", name="tp")

                # h0 transposes -> f32r lhsT tiles
                h0T = {}
                for l in (1, 2):
                    pt = tp_tile()
                    nc.tensor.transpose(pt[:, 0:32], h0b[l][:, 0:128],
                                        idn[0:32, 0:32])
                    nc.tensor.transpose(pt[:, 32:64], h0b[l][:, 128:256],
                                        idn[0:32, 0:32])
                    ht = cp.tile([128, 64], f32r, tag=f"h0T{l}",
                                 name=f"h0T{l}")
                    nc.vector.tensor_copy(ht[:], pt[:, 0:64])
                    h0T[l] = ht

                def make_xcatT(c):
                    xc = sp2.tile([128, DIN], f32, tag="xcat", name="xcat")
                    nc.gpsimd.indirect_dma_start(
                        out=xc[:, 0:E], out_offset=None, in_=embed[:],
                        in_offset=bass.IndirectOffsetOnAxis(
                            ap=loc_sb[:, c:c + 1], axis=0),
                    )
                    nc.vector.tensor_scalar(
                        out=xc[:, E:DIN], in0=iota_f[:, 0:TD],
                        scalar1=tim_f[:, c:c + 1], scalar2=None,
                        op0=ALU.is_equal,
                    )
                    pt = tp_tile()
                    nc.tensor.transpose(pt[:, 0:128], xc[:, 0:128],
                                        idn[0:128, 0:128])
                    nc.tensor.transpose(pt[0:48, 128:256], xc[:, 128:DIN],
                                        idn[0:128, 0:128])
                    xa = sp3.tile([128, 128], f32r, tag="xcatTa",
                                  name="xcatTa")
                    xb = sp3.tile([48, 128], f32r, tag="xcatTb", name="xcatTb")
                    nc.vector.tensor_copy(xa[:], pt[:, 0:128])
                    nc.vector.tensor_copy(xb[:], pt[0:48, 128:256])
                    return xa, xb

                def gru_step(l, t, xA, xB, wxa, wxb, hprev_b, hTa, hTb):
                    """One GRU step, [32, *] batch-on-partition layout."""
                    wa, wb = whh[l]
                    hp = psh.tile([32, G], f32, tag="hp", name="hp")
                    # r+z: one N=512 accumulation group (x-proj + bias + h)
                    nc.tensor.matmul(hp[:, 0:512], xA, wxa[:, 0:512],
                                     start=True, stop=False)
                    nc.tensor.matmul(hp[:, 0:512], xB, wxb[:, 0:512],
                                     start=False, stop=False)
                    if gru_bias_nz:
                        nc.tensor.matmul(hp[:, 0:512], ones32r[:1, :],
                                         brz[l][:1, :], start=False,
                                         stop=False)
                    nc.tensor.matmul(hp[:, 0:512], hTa, wa[:, 0:512],
                                     start=False, stop=False)
                    nc.tensor.matmul(hp[:, 0:512], hTb, wb[:, 0:512],
                                     start=False, stop=True)
                    # hn
                    nc.tensor.matmul(hp[:, 512:G], hTa, wa[:, 512:G],
                                     start=True, stop=False)
                    nc.tensor.matmul(hp[:, 512:G], hTb, wb[:, 512:G],
                                     start=False, stop=not gru_bias_nz)
                    if gru_bias_nz:
                        nc.tensor.matmul(hp[:, 512:G], ones32r[:1, :],
                                         bhn[l][:1, :], start=False, stop=True)
                    # xn
                    xn = psn.tile([32, 256], f32, tag="xn", name="xn")
                    nc.tensor.matmul(xn[:], xA, wxa[:, 512:G],
                                     start=True, stop=False)
                    nc.tensor.matmul(xn[:], xB, wxb[:, 512:G],
                                     start=False, stop=not gru_bias_nz)
                    if gru_bias_nz:
                        nc.tensor.matmul(xn[:], ones32r[:1, :], bxn[l][:1, :],
                                         start=False, stop=True)
                    s = sp2.tile([32, G], f32, tag=f"s{l}", name=f"s{l}")
                    g = sp2.tile([32, G], f32, tag=f"g{l}", name=f"g{l}")
                    # r,z: sigmoid(x) = 0.5*tanh(x/2) + 0.5 — keeps the whole
                    # kernel inside the Tanh+Exp ACT table set (no reloads)
                    nc.scalar.activation(g[:, 0:512], hp[:, 0:512], AF.Tanh,
                                         scale=0.5)
                    nc.vector.tensor_scalar(
                        out=g[:, 0:512], in0=g[:, 0:512], scalar1=0.5,
                        scalar2=0.5, op0=ALU.mult, op1=ALU.add)
                    # n = tanh(xn + r*hn)
                    nc.vector.tensor_tensor(out=s[:, 512:G], in0=g[:, 0:256],
                                            in1=hp[:, 512:G], op=ALU.mult)
                    nc.vector.tensor_tensor(out=s[:, 512:G], in0=s[:, 512:G],
                                            in1=xn[:], op=ALU.add)
                    nc.scalar.activation(g[:, 512:G], s[:, 512:G], AF.Tanh)
                    # h_new = n + z*(h_prev - n)
                    nc.vector.tensor_tensor(out=s[:, 0:256], in0=hprev_b,
                                            in1=g[:, 512:G], op=ALU.subtract)
                    nc.vector.tensor_tensor(out=s[:, 256:512],
                                            in0=g[:, 256:512],
                                            in1=s[:, 0:256], op=ALU.mult)
                    hn = sp2.tile([32, H], f32, tag=f"h{l}", name=f"h{l}")
                    nc.vector.tensor_tensor(out=hn[:], in0=g[:, 512:G],
                                            in1=s[:, 256:512], op=ALU.add)
                    # transpose h_new for the next step's lhsT (+ storage)
                    pt = tp_tile()
                    nc.tensor.transpose(pt[:, 0:32], hn[:, 0:128],
                                        idn[0:32, 0:32])
                    nc.tensor.transpose(pt[:, 32:64], hn[:, 128:256],
                                        idn[0:32, 0:32])
                    pt2 = pt[:, 0:64].rearrange("p (k r) -> p k r", k=2)
                    if l == 1:
                        dst = y1T.rearrange("p (k r) -> p k r",
                                            k=2)[:, :, 32 * t:32 * t + 32]
                        nc.vector.tensor_copy(dst, pt2)
                        nTa = y1T[:, 32 * t:32 * t + 32]
                        nTb = y1T[:, R + 32 * t:R + 32 * t + 32]
                    else:
                        ht = sp2.tile([128, 64], f32r, tag="h2T", name="h2T")
                        nc.vector.tensor_copy(ht[:], pt[:, 0:64])
                        rdst = yrT.rearrange("p (k r) -> p k r",
                                             k=2)[:, :, 32 * t:32 * t + 32]
                        nc.scalar.activation(rdst, pt2, AF.Relu)
                        nTa, nTb = ht[:, 0:32], ht[:, 32:64]
                    return hn, nTa, nTb

                # FC pass-1 quarter-chunks: compute sum(exp(x+b)) per row.
                # (The time head and all Ln work run post-scan so the scan
                # window stays inside one ACT table set.)
                NPARTS = ((0, 1, 2), (3, 4, 5), (6, 7), (8, 9))
                accn_tiles = {}

                def fc_mm(pp, c, n0, n1):
                    nc.tensor.matmul(pp[:, 0:n1 - n0],
                                     yrT[:, 128 * c:128 * c + 128],
                                     fcwa[:, n0:n1], start=True, stop=False)
                    nc.tensor.matmul(pp[:, 0:n1 - n0],
                                     yrT[:, R + 128 * c:R + 128 * c + 128],
                                     fcwb[:, n0:n1], start=False, stop=True)

                def pass1_part(c, part):
                    if part == 0:
                        accn_tiles[c] = fc3.tile([128, NCH], f32, tag="accn",
                                                 name="accn")
                    accn = accn_tiles[c]
                    for n in NPARTS[part]:
                        pp = fp.tile([128, 512], f32, tag="fc", name="fc")
                        fc_mm(pp, c, NW * n, NW * (n + 1))
                        es = fc3.tile([128, NW], f32, tag="exps", name="exps")
                        if fcb_nz:
                            nc.vector.tensor_tensor(
                                out=es[:], in0=pp[:, 0:NW],
                                in1=fcb_all[:, NW * n:NW * (n + 1)],
                                op=ALU.add)
                            nc.scalar.activation(
                                es[:], es[:], AF.Exp,
                                accum_out=accn[:, n:n + 1])
                        else:
                            nc.scalar.activation(
                                es[:], pp[:, 0:NW], AF.Exp,
                                accum_out=accn[:, n:n + 1])
                    if part == 3:
                        nc.vector.tensor_reduce(
                            accs[:, c:c + 1], accn[:], axis=AX.X, op=ALU.add)

                def launch_ar(g):
                    nc.sync.dma_start(cc_in[g][:], accs[:, 4 * g:4 * g + 4])
                    nc.gpsimd.collective_compute(
                        "AllReduce", ALU.add,
                        replica_groups=[list(range(NCORES))],
                        ins=[cc_in[g].opt()], outs=[cc_out[g].opt()],
                    )

                def collect_group(g):
                    nc.sync.dma_start(rsums[:, 4 * g:4 * g + 4],
                                      cc_out[g][:])
                    nc.scalar.activation(nlz[:, 4 * g:4 * g + 4],
                                         rsums[:, 4 * g:4 * g + 4], AF.Ln)
                    nc.vector.tensor_scalar_mul(nlz[:, 4 * g:4 * g + 4],
                                                nlz[:, 4 * g:4 * g + 4], -1.0)

                def pass2_chunk(c, fop):
                    fo = fop.tile([128, VS], f32, tag="fout", name="fout")
                    for n in range(NCH):
                        pp = fp.tile([128, 512], f32, tag="fc", name="fc")
                        fc_mm(pp, c, NW * n, NW * (n + 1))
                        fs_ = fo[:, NW * n:NW * (n + 1)]
                        if fcb_nz:
                            nc.vector.scalar_tensor_tensor(
                                out=fs_, in0=pp[:, 0:NW],
                                scalar=nlz[:, c:c + 1],
                                in1=fcb_all[:, NW * n:NW * (n + 1)],
                                op0=ALU.add, op1=ALU.add)
                        elif n % 2 == 0:
                            nc.vector.tensor_scalar(
                                out=fs_, in0=pp[:, 0:NW],
                                scalar1=nlz[:, c:c + 1],
                                scalar2=None, op0=ALU.add)
                        else:
                            nc.scalar.activation(fs_, pp[:, 0:NW],
                                                 AF.Identity,
                                                 bias=nlz[:, c:c + 1])
                    nc.sync.dma_start(
                        bass.AP(out_loc, 128 * c * VS, [[VS, 128], [1, VS]]),
                        fo[:])

                # ---------- the scan loop ----------
                xcatT = {0: make_xcatT(0), 1: make_xcatT(1)}
                h1b, h2b = h0b[1][:, :], h0b[2][:, :]
                h1Ta, h1Tb = h0T[1][:, 0:32], h0T[1][:, 32:64]
                h2Ta, h2Tb = h0T[2][:, 0:32], h0T[2][:, 32:64]

                def l2_step(t2, h2b, h2Ta, h2Tb):
                    return gru_step(2, t2,
                                    y1T[:, 32 * t2:32 * t2 + 32],
                                    y1T[:, R + 32 * t2:R + 32 * t2 + 32],
                                    wih1a, wih1b, h2b, h2Ta, h2Tb)

                with tc.tile_pool(name="fout_p", bufs=2) as fop:
                    for t in range(T):
                        if t % 4 == 0 and t // 4 + 2 < RC:
                            xcatT[t // 4 + 2] = make_xcatT(t // 4 + 2)
                        xa, xb = xcatT[t // 4]
                        tau = 32 * (t % 4)
                        hn, h1Ta, h1Tb = gru_step(
                            1, t, xa[:, tau:tau + 32], xb[:, tau:tau + 32],
                            wih0a, wih0b, h1b, h1Ta, h1Tb)
                        h1b = hn[:]
                        if t >= 1:
                            hn2, h2Ta, h2Tb = l2_step(t - 1, h2b, h2Ta, h2Tb)
                            h2b = hn2[:]
                        # interleave FC pass 1 for finished row chunks:
                        # chunk c is done after layer-2 step 4c+3 (wave 4c+4)
                        if t >= 4:
                            c, part = (t - 4) // 4, (t - 4) % 4
                            if c < RC - 1:
                                pass1_part(c, part)
                        if t == 20:
                            launch_ar(0)
                        if t == 36:
                            launch_ar(1)
                        # stream the first pass-2 chunks into the late scan
                        # (their AllReduce finished ~15 waves after launch)
                        if t == 38:
                            collect_group(0)
                        if t in (39, 42, 45):
                            pass2_chunk((t - 39) // 3, fop)

                    hn2, h2Ta, h2Tb = l2_step(T - 1, h2b, h2Ta, h2Tb)
                    for part in range(4):
                        pass1_part(RC - 1, part)
                    launch_ar(2)
                    collect_group(1)

                    # time head: local 48-wide log-softmax (post-scan; needs
                    # the Ln table set)
                    for c in range(RC):
                        pp = fp.tile([128, 512], f32, tag="fc", name="fc")
                        fc_mm(pp, c, VS, VTOT)
                        tme = fc3.tile([128, TD], f32, tag="tme", name="tme")
                        tma = fc3.tile([128, 1], f32, tag="tma", name="tma")
                        if fcb_nz:
                            nc.vector.tensor_tensor(
                                out=tme[:], in0=pp[:, 0:TD],
                                in1=fcb_all[:, VS:VTOT], op=ALU.add)
                            nc.scalar.activation(tme[:], tme[:], AF.Exp,
                                                 accum_out=tma[:])
                        else:
                            nc.scalar.activation(tme[:], pp[:, 0:TD], AF.Exp,
                                                 accum_out=tma[:])
                        tml = fc3.tile([128, 1], f32, tag="tml", name="tml")
                        nc.scalar.activation(tml[:], tma[:], AF.Ln)
                        nc.vector.tensor_scalar_mul(tml[:], tml[:], -1.0)
                        tmo = tm_all[:, TD * c:TD * (c + 1)]
                        if fcb_nz:
                            nc.vector.scalar_tensor_tensor(
                                out=tmo, in0=pp[:, 0:TD], scalar=tml[:, :1],
                                in1=fcb_all[:, VS:VTOT],
                                op0=ALU.add, op1=ALU.add)
                        else:
                            nc.vector.tensor_scalar(
                                out=tmo, in0=pp[:, 0:TD], scalar1=tml[:, :1],
                                scalar2=None, op0=ALU.add)

                    # remaining pass-2 chunks
                    for c in range(3, RC):
                        if c == 8:
                            collect_group(2)
                        pass2_chunk(c, fop)

                    # single DMA for the whole time-head output (t-major)
                    nc.sync.dma_start(
                        bass.AP(out_tm, 0, [[TD, 128], [128 * TD, RC], [1, TD]]),
                        tm_all[:])

    nc.compile()
    return nc


def _get_nc(gru_bias_nz, fcb_nz):
    key = (gru_bias_nz, fcb_nz)
    if key not in _cache:
        _cache[key] = _build(*key)
    return _cache[key]


def _prep_inputs(locations, times, labels, embed_table, traj_table, fc_w, fc_b,
                 w_ih0, w_hh0, b_ih0, b_hh0, w_ih1, w_hh1, b_ih1, b_hh1):
    f = np.float32
    locations = np.asarray(locations)
    times = np.asarray(times)
    labels = np.asarray(labels)
    # t-major row layout: r = t*32 + b; [128, RC] with [p, c] = row 128c+p
    loc_tm = np.ascontiguousarray(
        locations.T.reshape(RC, 128).T).astype(np.int32)
    tim_tm = np.ascontiguousarray(
        times.T.reshape(RC, 128).T).astype(np.int32)
    # h0 strip-gather indices (torch .view(L, -1, H) semantics):
    # h0[l, b] = traj_table.view(20, 256)[2*labels[16l + b//2] + b%2]
    p = np.arange(2 * B)
    l_, b_ = p // B, p % B
    lab_ix = (2 * labels[(B // 2) * l_ + b_ // 2] + b_ % 2).astype(np.int32)
    lab_ix = np.ascontiguousarray(lab_ix.reshape(2 * B, 1))

    common = dict(
        loc_tm=loc_tm, tim_tm=tim_tm, lab_ix=lab_ix,
        embed=np.ascontiguousarray(embed_table, dtype=f),
        traj=np.ascontiguousarray(
            np.asarray(traj_table, dtype=f).reshape(2 * TRAJ, H)),
        wih0T=np.ascontiguousarray(np.asarray(w_ih0, dtype=f).T),
        wih1T=np.ascontiguousarray(np.asarray(w_ih1, dtype=f).T),
        whh0T=np.ascontiguousarray(np.asarray(w_hh0, dtype=f).T),
        whh1T=np.ascontiguousarray(np.asarray(w_hh1, dtype=f).T),
    )
    b_ih0 = np.asarray(b_ih0, dtype=f)
    b_hh0 = np.asarray(b_hh0, dtype=f)
    b_ih1 = np.asarray(b_ih1, dtype=f)
    b_hh1 = np.asarray(b_hh1, dtype=f)
    gru_bias_nz = bool(np.any(b_ih0) or np.any(b_hh0) or np.any(b_ih1)
                       or np.any(b_hh1))
    if gru_bias_nz:
        common["gb0"] = np.ascontiguousarray(np.stack([b_ih0, b_hh0]))
        common["gb1"] = np.ascontiguousarray(np.stack([b_ih1, b_hh1]))

    fc_w = np.asarray(fc_w, dtype=f)
    fc_b = np.asarray(fc_b, dtype=f)
    fcb_nz = bool(np.any(fc_b))

    in_maps = []
    for c in range(NCORES):
        m = dict(common)
        wslice = np.concatenate([fc_w[c * VS:(c + 1) * VS], fc_w[OUT:]],
                                axis=0)
        m["fcwT"] = np.ascontiguousarray(wslice.T)
        if fcb_nz:
            bslice = np.concatenate([fc_b[c * VS:(c + 1) * VS], fc_b[OUT:]])
            m["fcb"] = np.ascontiguousarray(bslice.reshape(1, VTOT))
        in_maps.append(m)
    return in_maps, gru_bias_nz, fcb_nz


def _run(in_maps, gru_bias_nz, fcb_nz, trace=False):
    from concourse.bass_utils import run_bass_kernel_spmd
    nc = _get_nc(gru_bias_nz, fcb_nz)
    if trace:
        import sys as _sys
        import types as _types
        try:
            from antenv.axon_hooks import get_axon_ntff_profile_hook  # noqa
        except ImportError:
            from trn_agent_boot.trn_boot import _ntff_profile_via_ctypes
            _h = _ntff_profile_via_ctypes('/opt/axon/libaxon_pjrt.so')
            _m = _types.ModuleType('antenv.axon_hooks')
            _m.get_axon_ntff_profile_hook = lambda: _h
            _m.set_axon_ntff_profile_hook = lambda h: None
            _sys.modules['antenv.axon_hooks'] = _m
    return run_bass_kernel_spmd(nc, in_maps, list(range(NCORES)), trace=trace)


def _assemble(results):
    # device outputs are t-major [R, *] with r = t*32 + b
    loc = np.empty((B, T, OUT), np.float32)
    for c in range(NCORES):
        loc[:, :, c * VS:(c + 1) * VS] = (
            results[c]["out_loc"].reshape(T, B, VS).transpose(1, 0, 2))
    tm = np.ascontiguousarray(
        results[0]["out_tm"].reshape(T, B, TD).transpose(1, 0, 2))
    return loc, tm


def kernel(**inputs):
    in_maps, gru_bias_nz, fcb_nz = _prep_inputs(**inputs)
    res = _run(in_maps, gru_bias_nz, fcb_nz, trace=False)
    return _assemble(res.results)


# revision 36
# speedup vs baseline: 1.7485x; 1.1814x over previous
"""Trainium2 Bass kernel for nn_MetaDiscreteTimeTrajTypeGRUNet.

Strategy (8 NeuronCores, SPMD):
  - GRU (2 layers, T=48, B=32) replicated on all cores (latency-bound, tiny).
    Wavefront schedule: layer 2 runs 1 step behind layer 1; the input
    projections (x@w_ih) are fused into the recurrent PSUM accumulation
    groups, so there is no separate xp precompute.
  - FC + log_softmax vocab-parallel: each core owns 5000 output columns
    (+ every core computes the small 48-col time head; core 0's is used).
  - log_softmax without max-subtraction (logits are O(1) by construction):
    pass 1 (interleaved into the scan as row chunks finish) computes
    sum(exp(x+b)) per row; three staged 2KB AllReduces combine the 8 vocab
    shards (the first two complete during the scan); pass 2 recomputes the
    matmul (weights stay in SBUF) and writes x + b - log(Z) with one
    batched 2.5MB DMA per row chunk.
  - All matmuls in float32r (full PE rate at N>=256).

Row indexing on device is t-major: r = t*32 + b; chunk c = rows 128c..128c+127
(= timesteps 4c..4c+3, all 32 batch elements each). DRAM outputs are written
b-major via strided 3D access patterns.
"""
import numpy as np

B, T, E, TD = 32, 48, 128, 48
H, OUT, TRAJ = 256, 40000, 10
DIN = E + TD          # 176
G = 3 * H             # 768
NCORES = 8
VS = OUT // NCORES    # 5000 vocab columns per core
VTOT = VS + TD        # 5048 fc columns per core (vocab slice + time head)
R = B * T             # 1536
RC = R // 12          # noqa: E501  (unused alias guard)
RC = R // 128         # 12 row chunks
NCH = 10              # vocab N-chunks per row chunk
NW = VS // NCH        # 500
NGRP = 3              # allreduce groups (4 row chunks each)

_cache = {}


def _build(gru_bias_nz: bool, fcb_nz: bool):
    import concourse.bass as bass
    import concourse.mybir as mybir
    import concourse.tile as tile
    import concourse.bacc as bacc
    from concourse.masks import make_identity

    f32 = mybir.dt.float32
    f32r = mybir.dt.float32r
    i32 = mybir.dt.int32
    AF = mybir.ActivationFunctionType
    ALU = mybir.AluOpType
    AX = mybir.AxisListType

    nc = bacc.Bacc("TRN2", target_bir_lowering=False, debug=False,
                   num_devices=NCORES)

    # ---------------- I/O ----------------
    loc_tm = nc.dram_tensor("loc_tm", [128, RC], i32, kind="ExternalInput")
    tim_tm = nc.dram_tensor("tim_tm", [128, RC], i32, kind="ExternalInput")
    lab_ix = nc.dram_tensor("lab_ix", [2 * B, 1], i32, kind="ExternalInput")
    embed = nc.dram_tensor("embed", [OUT, E], f32, kind="ExternalInput")
    traj = nc.dram_tensor("traj", [2 * TRAJ, H], f32, kind="ExternalInput")
    wih0T_d = nc.dram_tensor("wih0T", [DIN, G], f32, kind="ExternalInput")
    wih1T_d = nc.dram_tensor("wih1T", [H, G], f32, kind="ExternalInput")
    whh0T_d = nc.dram_tensor("whh0T", [H, G], f32, kind="ExternalInput")
    whh1T_d = nc.dram_tensor("whh1T", [H, G], f32, kind="ExternalInput")
    fcwT_d = nc.dram_tensor("fcwT", [H, VTOT], f32, kind="ExternalInput")
    if gru_bias_nz:
        gb0_d = nc.dram_tensor("gb0", [2, G], f32, kind="ExternalInput")
        gb1_d = nc.dram_tensor("gb1", [2, G], f32, kind="ExternalInput")
    if fcb_nz:
        fcb_d = nc.dram_tensor("fcb", [1, VTOT], f32, kind="ExternalInput")

    # outputs are written t-major (row r = t*32+b, contiguous 2.5MB blocks
    # per row chunk — ~3.4x the DMA bandwidth of b-major strided writes);
    # the host reindexes to [B, T, *]
    out_loc = nc.dram_tensor("out_loc", [R, VS], f32, kind="ExternalOutput")
    out_tm = nc.dram_tensor("out_tm", [R, TD], f32, kind="ExternalOutput")

    with tile.TileContext(nc) as tc:
        with (
            tc.tile_pool(name="const", bufs=1) as cp,
            tc.tile_pool(name="dram", bufs=1, space="DRAM") as dp,
        ):
            # ---------- constants ----------
            idn = cp.tile([128, 128], f32, tag="idn", name="idn")
            make_identity(nc, idn[:])

            iota_i = cp.tile([128, TD], i32, tag="iota_i", name="iota_i")
            nc.gpsimd.iota(iota_i[:], pattern=[[1, TD]], base=0,
                           channel_multiplier=0)
            iota_f = cp.tile([128, TD], f32, tag="iota_f", name="iota_f")
            nc.vector.tensor_copy(iota_f[:], iota_i[:])

            loc_sb = cp.tile([128, RC], i32, tag="loc_sb", name="loc_sb")
            nc.sync.dma_start(loc_sb[:], loc_tm[:])
            tim_i = cp.tile([128, RC], i32, tag="tim_i", name="tim_i")
            nc.sync.dma_start(tim_i[:], tim_tm[:])
            tim_f = cp.tile([128, RC], f32, tag="tim_f", name="tim_f")
            nc.vector.tensor_copy(tim_f[:], tim_i[:])

            # h0 gather early (overlaps the weight loads below)
            h0b = {}
            for l, li in ((1, 0), (2, 1)):
                ls = cp.tile([B, 1], i32, tag=f"lab{l}", name=f"lab{l}")
                nc.sync.dma_start(ls[:], lab_ix[32 * li:32 * li + 32, :])
                hb = cp.tile([B, H], f32, tag=f"h0b{l}", name=f"h0b{l}")
                nc.gpsimd.indirect_dma_start(
                    out=hb[:], out_offset=None, in_=traj[:],
                    in_offset=bass.IndirectOffsetOnAxis(ap=ls[:, :1], axis=0),
                )
                h0b[l] = hb

            # weights -> SBUF as f32r via staging + rounding copy
            # (scan weights first; the big fc slabs last)
            with tc.tile_pool(name="stage", bufs=2) as stp:
                def load_w(dram_h, r0, r1, cols, tag):
                    st = stp.tile([r1 - r0, cols], f32, tag="wstage",
                                  name="wstage")
                    nc.sync.dma_start(st[:], dram_h[r0:r1, :])
                    t = cp.tile([r1 - r0, cols], f32r, tag=tag, name=tag)
                    nc.vector.tensor_copy(t[:], st[:])
                    return t

                wih0a = load_w(wih0T_d, 0, 128, G, "wih0a")
                wih0b = load_w(wih0T_d, 128, DIN, G, "wih0b")
                wih1a = load_w(wih1T_d, 0, 128, G, "wih1a")
                wih1b = load_w(wih1T_d, 128, H, G, "wih1b")
                whh = {
                    1: (load_w(whh0T_d, 0, 128, G, "whh0a"),
                        load_w(whh0T_d, 128, H, G, "whh0b")),
                    2: (load_w(whh1T_d, 0, 128, G, "whh1a"),
                        load_w(whh1T_d, 128, H, G, "whh1b")),
                }
                fcwa = load_w(fcwT_d, 0, 128, VTOT, "fcwa")
                fcwb = load_w(fcwT_d, 128, H, VTOT, "fcwb")

            # transposed y storage, two K-halves side by side:
            # [:, 0:R] = K rows 0:128, [:, R:2R] = K rows 128:256
            y1T = cp.tile([128, 2 * R], f32r, tag="y1T", name="y1T")
            yrT = cp.tile([128, 2 * R], f32r, tag="yrT", name="yrT")

            # optional fc bias materialization via ones-matmul broadcast
            fcb_all = None
            if fcb_nz:
                ones_s = cp.tile([1, 128], f32, tag="ones_s", name="ones_s")
                nc.vector.memset(ones_s[:1, :], 1.0)
                ones_r = cp.tile([1, 128], f32r, tag="ones_r", name="ones_r")
                nc.vector.tensor_copy(ones_r[:1, :], ones_s[:1, :])
                with tc.tile_pool(name="bps", bufs=1, space="PSUM") as bps:
                    row_s = cp.tile([1, VTOT], f32, tag="fcb_row_s",
                                    name="fcb_row_s")
                    nc.sync.dma_start(row_s[:1, :], fcb_d[:])
                    row = cp.tile([1, VTOT], f32r, tag="fcb_row",
                                  name="fcb_row")
                    nc.vector.tensor_copy(row[:1, :], row_s[:1, :])
                    fcb_all = cp.tile([128, VTOT], f32, tag="fcb_full",
                                      name="fcb_full")
                    for n0 in range(0, VTOT, 512):
                        n1 = min(n0 + 512, VTOT)
                        pb = bps.tile([128, 512], f32, tag="bps", name="bps")
                        nc.tensor.matmul(pb[:, 0:n1 - n0], ones_r[:1, :],
                                         row[:1, n0:n1], start=True, stop=True)
                        nc.vector.tensor_copy(fcb_all[:, n0:n1],
                                              pb[:, 0:n1 - n0])

            # GRU bias rows (general path) as K=1 rank-1 matmul contributions
            brz = {}
            bhn = {}
            bxn = {}
            if gru_bias_nz:
                ones32 = cp.tile([1, 32], f32, tag="ones32", name="ones32")
                nc.vector.memset(ones32[:1, :], 1.0)
                ones32r = cp.tile([1, 32], f32r, tag="ones32r", name="ones32r")
                nc.vector.tensor_copy(ones32r[:1, :], ones32[:1, :])
                for l, gbd in ((1, gb0_d), (2, gb1_d)):
                    bi = cp.tile([1, G], f32, tag=f"bi{l}", name=f"bi{l}")
                    bh = cp.tile([1, G], f32, tag=f"bh{l}", name=f"bh{l}")
                    nc.sync.dma_start(bi[:1, :], gbd[0:1, :])
                    nc.sync.dma_start(bh[:1, :], gbd[1:2, :])
                    bs = cp.tile([1, 512], f32, tag=f"brzs{l}",
                                 name=f"brzs{l}")
                    nc.vector.tensor_tensor(out=bs[:1, :], in0=bi[:1, 0:512],
                                            in1=bh[:1, 0:512], op=ALU.add)
                    br = cp.tile([1, 512], f32r, tag=f"brz{l}", name=f"brz{l}")
                    nc.vector.tensor_copy(br[:1, :], bs[:1, :])
                    bn1 = cp.tile([1, 256], f32r, tag=f"bhn{l}",
                                  name=f"bhn{l}")
                    nc.vector.tensor_copy(bn1[:1, :], bh[:1, 512:G])
                    bn2 = cp.tile([1, 256], f32r, tag=f"bxn{l}",
                                  name=f"bxn{l}")
                    nc.vector.tensor_copy(bn2[:1, :], bi[:1, 512:G])
                    brz[l], bhn[l], bxn[l] = br, bn1, bn2

            # ---------- scan + interleaved FC pass 1 ----------
            with (
                tc.tile_pool(name="scan2", bufs=2) as sp2,
                tc.tile_pool(name="scan3", bufs=3) as sp3,
                tc.tile_pool(name="fcp1", bufs=1) as fc1,
                tc.tile_pool(name="fcp3", bufs=3) as fc3,
                tc.tile_pool(name="ps_fc", bufs=2, space="PSUM") as fp,
                tc.tile_pool(name="fout_p", bufs=2) as fop,
            ):
                accs = fc1.tile([128, RC], f32, tag="accs", name="accs")
                nlz = fc1.tile([128, RC], f32, tag="nlz", name="nlz")
                rsums = fc1.tile([128, RC], f32, tag="rsums", name="rsums")
                tm_all = fc1.tile([128, RC * TD], f32, tag="tm_all",
                                  name="tm_all")
                cc_in = [dp.tile([128, 4], f32, name=f"cc_in{g}")
                         for g in range(NGRP)]
                cc_out = [dp.tile([128, 4], f32, name=f"cc_out{g}")
                          for g in range(NGRP)]

                scan_ps = tc.tile_pool(name="ps_hp", bufs=2, space="PSUM")
                psh = scan_ps.__enter__()
                scan_ps2 = tc.tile_pool(name="ps_xn", bufs=1, space="PSUM")
                psn = scan_ps2.__enter__()
                scan_ps3 = tc.tile_pool(name="ps_tp", bufs=1, space="PSUM")
                pst = scan_ps3.__enter__()

                def tp_tile():
                    return pst.tile([128, 256], f32, tag="tp", name="tp")

                # h0 transposes -> f32r lhsT tiles
                h0T = {}
                for l in (1, 2):
                    pt = tp_tile()
                    nc.tensor.transpose(pt[:, 0:32], h0b[l][:, 0:128],
                                        idn[0:32, 0:32])
                    nc.tensor.transpose(pt[:, 32:64], h0b[l][:, 128:256],
                                        idn[0:32, 0:32])
                    ht = cp.tile([128, 64], f32r, tag=f"h0T{l}",
                                 name=f"h0T{l}")
                    nc.vector.tensor_copy(ht[:], pt[:, 0:64])
                    h0T[l] = ht

                def make_xcatT(c):
                    xc = sp2.tile([128, DIN], f32, tag="xcat", name="xcat")
                    nc.gpsimd.indirect_dma_start(
                        out=xc[:, 0:E], out_offset=None, in_=embed[:],
                        in_offset=bass.IndirectOffsetOnAxis(
                            ap=loc_sb[:, c:c + 1], axis=0),
                    )
                    nc.vector.tensor_scalar(
                        out=xc[:, E:DIN], in0=iota_f[:, 0:TD],
                        scalar1=tim_f[:, c:c + 1], scalar2=None,
                        op0=ALU.is_equal,
                    )
                    pt = tp_tile()
                    nc.tensor.transpose(pt[:, 0:128], xc[:, 0:128],
                                        idn[0:128, 0:128])
                    nc.tensor.transpose(pt[0:48, 128:256], xc[:, 128:DIN],
                                        idn[0:128, 0:128])
                    xa = sp3.tile([128, 128], f32r, tag="xcatTa",
                                  name="xcatTa")
                    xb = sp3.tile([48, 128], f32r, tag="xcatTb", name="xcatTb")
                    nc.vector.tensor_copy(xa[:], pt[:, 0:128])
                    nc.vector.tensor_copy(xb[:], pt[0:48, 128:256])
                    return xa, xb

                def gru_step(l, t, xA, xB, wxa, wxb, hprev_b, hTa, hTb):
                    """One GRU step, [32, *] batch-on-partition layout."""
                    wa, wb = whh[l]
                    hp = psh.tile([32, G], f32, tag="hp", name="hp")
                    # r+z: one N=512 accumulation group (x-proj + bias + h)
                    nc.tensor.matmul(hp[:, 0:512], xA, wxa[:, 0:512],
                                     start=True, stop=False)
                    nc.tensor.matmul(hp[:, 0:512], xB, wxb[:, 0:512],
                                     start=False, stop=False)
                    if gru_bias_nz:
                        nc.tensor.matmul(hp[:, 0:512], ones32r[:1, :],
                                         brz[l][:1, :], start=False,
                                         stop=False)
                    nc.tensor.matmul(hp[:, 0:512], hTa, wa[:, 0:512],
                                     start=False, stop=False)
                    nc.tensor.matmul(hp[:, 0:512], hTb, wb[:, 0:512],
                                     start=False, stop=True)
                    # hn
                    nc.tensor.matmul(hp[:, 512:G], hTa, wa[:, 512:G],
                                     start=True, stop=False)
                    nc.tensor.matmul(hp[:, 512:G], hTb, wb[:, 512:G],
                                     start=False, stop=not gru_bias_nz)
                    if gru_bias_nz:
                        nc.tensor.matmul(hp[:, 512:G], ones32r[:1, :],
                                         bhn[l][:1, :], start=False, stop=True)
                    # xn
                    xn = psn.tile([32, 256], f32, tag="xn", name="xn")
                    nc.tensor.matmul(xn[:], xA, wxa[:, 512:G],
                                     start=True, stop=False)
                    nc.tensor.matmul(xn[:], xB, wxb[:, 512:G],
                                     start=False, stop=not gru_bias_nz)
                    if gru_bias_nz:
                        nc.tensor.matmul(xn[:], ones32r[:1, :], bxn[l][:1, :],
                                         start=False, stop=True)
                    s = sp2.tile([32, G], f32, tag=f"s{l}", name=f"s{l}")
                    g = sp2.tile([32, G], f32, tag=f"g{l}", name=f"g{l}")
                    # r,z: sigmoid(x) = 0.5*tanh(x/2) + 0.5 — keeps the whole
                    # kernel inside the Tanh+Exp ACT table set (no reloads)
                    nc.scalar.activation(g[:, 0:512], hp[:, 0:512], AF.Tanh,
                                         scale=0.5)
                    nc.vector.tensor_scalar(
                        out=g[:, 0:512], in0=g[:, 0:512], scalar1=0.5,
                        scalar2=0.5, op0=ALU.mult, op1=ALU.add)
                    # n = tanh(xn + r*hn)
                    nc.vector.tensor_tensor(out=s[:, 512:G], in0=g[:, 0:256],
                                            in1=hp[:, 512:G], op=ALU.mult)
                    nc.vector.tensor_tensor(out=s[:, 512:G], in0=s[:, 512:G],
                                            in1=xn[:], op=ALU.add)
                    nc.scalar.activation(g[:, 512:G], s[:, 512:G], AF.Tanh)
                    # h_new = n + z*(h_prev - n)
                    nc.vector.tensor_tensor(out=s[:, 0:256], in0=hprev_b,
                                            in1=g[:, 512:G], op=ALU.subtract)
                    nc.vector.tensor_tensor(out=s[:, 256:512],
                                            in0=g[:, 256:512],
                                            in1=s[:, 0:256], op=ALU.mult)
                    hn = sp2.tile([32, H], f32, tag=f"h{l}", name=f"h{l}")
                    nc.vector.tensor_tensor(out=hn[:], in0=g[:, 512:G],
                                            in1=s[:, 256:512], op=ALU.add)
                    # transpose h_new for the next step's lhsT (+ storage)
                    pt = tp_tile()
                    nc.tensor.transpose(pt[:, 0:32], hn[:, 0:128],
                                        idn[0:32, 0:32])
                    nc.tensor.transpose(pt[:, 32:64], hn[:, 128:256],
                                        idn[0:32, 0:32])
                    pt2 = pt[:, 0:64].rearrange("p (k r) -> p k r", k=2)
                    if l == 1:
                        dst = y1T.rearrange("p (k r) -> p k r",
                                            k=2)[:, :, 32 * t:32 * t + 32]
                        nc.vector.tensor_copy(dst, pt2)
                        nTa = y1T[:, 32 * t:32 * t + 32]
                        nTb = y1T[:, R + 32 * t:R + 32 * t + 32]
                    else:
                        ht = sp2.tile([128, 64], f32r, tag="h2T", name="h2T")
                        nc.vector.tensor_copy(ht[:], pt[:, 0:64])
                        rdst = yrT.rearrange("p (k r) -> p k r",
                                             k=2)[:, :, 32 * t:32 * t + 32]
                        nc.scalar.activation(rdst, pt2, AF.Relu)
                        nTa, nTb = ht[:, 0:32], ht[:, 32:64]
                    return hn, nTa, nTb

                # FC pass-1 quarter-chunks: compute sum(exp(x+b)) per row.
                # (The time head and all Ln work run post-scan so the scan
                # window stays inside one ACT table set.)
                NPARTS = ((0, 1, 2), (3, 4, 5), (6, 7), (8, 9))
                accn_tiles = {}

                def fc_mm(pp, c, n0, n1):
                    nc.tensor.matmul(pp[:, 0:n1 - n0],
                                     yrT[:, 128 * c:128 * c + 128],
                                     fcwa[:, n0:n1], start=True, stop=False)
                    nc.tensor.matmul(pp[:, 0:n1 - n0],
                                     yrT[:, R + 128 * c:R + 128 * c + 128],
                                     fcwb[:, n0:n1], start=False, stop=True)

                def pass1_part(c, part):
                    if part == 0:
                        accn_tiles[c] = fc3.tile([128, NCH], f32, tag="accn",
                                                 name="accn")
                    accn = accn_tiles[c]
                    for n in NPARTS[part]:
                        pp = fp.tile([128, 500], f32, tag="fc", name="fc")
                        fc_mm(pp, c, NW * n, NW * (n + 1))
                        es = fc3.tile([128, NW], f32, tag="exps", name="exps")
                        if fcb_nz:
                            nc.vector.tensor_tensor(
                                out=es[:], in0=pp[:, 0:NW],
                                in1=fcb_all[:, NW * n:NW * (n + 1)],
                                op=ALU.add)
                            nc.scalar.activation(
                                es[:], es[:], AF.Exp,
                                accum_out=accn[:, n:n + 1])
                        else:
                            nc.scalar.activation(
                                es[:], pp[:, 0:NW], AF.Exp,
                                accum_out=accn[:, n:n + 1])
                    if part == 3:
                        nc.vector.tensor_reduce(
                            accs[:, c:c + 1], accn[:], axis=AX.X, op=ALU.add)

                def launch_ar(g):
                    nc.sync.dma_start(cc_in[g][:], accs[:, 4 * g:4 * g + 4])
                    nc.gpsimd.collective_compute(
                        "AllReduce", ALU.add,
                        replica_groups=[list(range(NCORES))],
                        ins=[cc_in[g].opt()], outs=[cc_out[g].opt()],
                    )

                def collect_group(g):
                    nc.sync.dma_start(rsums[:, 4 * g:4 * g + 4],
                                      cc_out[g][:])
                    nc.scalar.activation(nlz[:, 4 * g:4 * g + 4],
                                         rsums[:, 4 * g:4 * g + 4], AF.Ln)
                    nc.vector.tensor_scalar_mul(nlz[:, 4 * g:4 * g + 4],
                                                nlz[:, 4 * g:4 * g + 4], -1.0)

                def pass2_chunk(c, fcp):
                    fo = fop.tile([128, VS], f32, tag="fout", name="fout")
                    for n in range(NCH):
                        pp = fcp.tile([128, 500], f32, tag="fc", name="fc")
                        fc_mm(pp, c, NW * n, NW * (n + 1))
                        fs_ = fo[:, NW * n:NW * (n + 1)]
                        if fcb_nz:
                            nc.vector.scalar_tensor_tensor(
                                out=fs_, in0=pp[:, 0:NW],
                                scalar=nlz[:, c:c + 1],
                                in1=fcb_all[:, NW * n:NW * (n + 1)],
                                op0=ALU.add, op1=ALU.add)
                        elif n % 2 == 0:
                            nc.vector.tensor_scalar(
                                out=fs_, in0=pp[:, 0:NW],
                                scalar1=nlz[:, c:c + 1],
                                scalar2=None, op0=ALU.add)
                        else:
                            nc.scalar.activation(fs_, pp[:, 0:NW],
                                                 AF.Identity,
                                                 bias=nlz[:, c:c + 1])
                    nc.sync.dma_start(
                        bass.AP(out_loc, 128 * c * VS, [[VS, 128], [1, VS]]),
                        fo[:])

                # ---------- the scan loop ----------
                xcatT = {0: make_xcatT(0), 1: make_xcatT(1)}
                h1b, h2b = h0b[1][:, :], h0b[2][:, :]
                h1Ta, h1Tb = h0T[1][:, 0:32], h0T[1][:, 32:64]
                h2Ta, h2Tb = h0T[2][:, 0:32], h0T[2][:, 32:64]

                def l2_step(t2, h2b, h2Ta, h2Tb):
                    return gru_step(2, t2,
                                    y1T[:, 32 * t2:32 * t2 + 32],
                                    y1T[:, R + 32 * t2:R + 32 * t2 + 32],
                                    wih1a, wih1b, h2b, h2Ta, h2Tb)

                if True:
                    for t in range(T):
                        if t % 4 == 0 and t // 4 + 2 < RC:
                            xcatT[t // 4 + 2] = make_xcatT(t // 4 + 2)
                        xa, xb = xcatT[t // 4]
                        tau = 32 * (t % 4)
                        hn, h1Ta, h1Tb = gru_step(
                            1, t, xa[:, tau:tau + 32], xb[:, tau:tau + 32],
                            wih0a, wih0b, h1b, h1Ta, h1Tb)
                        h1b = hn[:]
                        if t >= 1:
                            hn2, h2Ta, h2Tb = l2_step(t - 1, h2b, h2Ta, h2Tb)
                            h2b = hn2[:]
                        # interleave FC pass 1 for finished row chunks:
                        # chunk c is done after layer-2 step 4c+3 (wave 4c+4)
                        if t >= 4:
                            c, part = (t - 4) // 4, (t - 4) % 4
                            if c < RC - 1:
                                pass1_part(c, part)
                        if t == 20:
                            launch_ar(0)
                        if t == 36:
                            launch_ar(1)
                        # stream the first pass-2 chunks into the late scan
                        # (their AllReduce finished ~15 waves after launch)
                        if t == 38:
                            collect_group(0)
                        if t in (39, 42, 45):
                            pass2_chunk((t - 39) // 3, fp)

                    hn2, h2Ta, h2Tb = l2_step(T - 1, h2b, h2Ta, h2Tb)
                    for part in range(4):
                        pass1_part(RC - 1, part)
                    scan_ps3.__exit__(None, None, None)
                    scan_ps2.__exit__(None, None, None)
                    scan_ps.__exit__(None, None, None)
                    launch_ar(2)
                    collect_group(1)

                    # time head: local 48-wide log-softmax (post-scan; needs
                    # the Ln table set)
                    for c in range(RC):
                        pp = fp.tile([128, 500], f32, tag="fc", name="fc")
                        fc_mm(pp, c, VS, VTOT)
                        tme = fc3.tile([128, TD], f32, tag="tme", name="tme")
                        tma = fc3.tile([128, 1], f32, tag="tma", name="tma")
                        if fcb_nz:
                            nc.vector.tensor_tensor(
                                out=tme[:], in0=pp[:, 0:TD],
                                in1=fcb_all[:, VS:VTOT], op=ALU.add)
                            nc.scalar.activation(tme[:], tme[:], AF.Exp,
                                                 accum_out=tma[:])
                        else:
                            nc.scalar.activation(tme[:], pp[:, 0:TD], AF.Exp,
                                                 accum_out=tma[:])
                        tml = fc3.tile([128, 1], f32, tag="tml", name="tml")
                        nc.scalar.activation(tml[:], tma[:], AF.Ln)
                        nc.vector.tensor_scalar_mul(tml[:], tml[:], -1.0)
                        tmo = tm_all[:, TD * c:TD * (c + 1)]
                        if fcb_nz:
                            nc.vector.scalar_tensor_tensor(
                                out=tmo, in0=pp[:, 0:TD], scalar=tml[:, :1],
                                in1=fcb_all[:, VS:VTOT],
                                op0=ALU.add, op1=ALU.add)
                        else:
                            nc.vector.tensor_scalar(
                                out=tmo, in0=pp[:, 0:TD], scalar1=tml[:, :1],
                                scalar2=None, op0=ALU.add)

                    # remaining pass-2 chunks with a deep psum pipeline
                    with tc.tile_pool(name="ps_fc2", bufs=5,
                                      space="PSUM") as fp2:
                        for c in range(3, RC):
                            if c == 8:
                                collect_group(2)
                            pass2_chunk(c, fp2)

                    # single DMA for the whole time-head output (t-major)
                    nc.sync.dma_start(
                        bass.AP(out_tm, 0, [[TD, 128], [128 * TD, RC], [1, TD]]),
                        tm_all[:])

    nc.compile()
    return nc


def _get_nc(gru_bias_nz, fcb_nz):
    key = (gru_bias_nz, fcb_nz)
    if key not in _cache:
        _cache[key] = _build(*key)
    return _cache[key]


def _prep_inputs(locations, times, labels, embed_table, traj_table, fc_w, fc_b,
                 w_ih0, w_hh0, b_ih0, b_hh0, w_ih1, w_hh1, b_ih1, b_hh1):
    f = np.float32
    locations = np.asarray(locations)
    times = np.asarray(times)
    labels = np.asarray(labels)
    # t-major row layout: r = t*32 + b; [128, RC] with [p, c] = row 128c+p
    loc_tm = np.ascontiguousarray(
        locations.T.reshape(RC, 128).T).astype(np.int32)
    tim_tm = np.ascontiguousarray(
        times.T.reshape(RC, 128).T).astype(np.int32)
    # h0 strip-gather indices (torch .view(L, -1, H) semantics):
    # h0[l, b] = traj_table.view(20, 256)[2*labels[16l + b//2] + b%2]
    p = np.arange(2 * B)
    l_, b_ = p // B, p % B
    lab_ix = (2 * labels[(B // 2) * l_ + b_ // 2] + b_ % 2).astype(np.int32)
    lab_ix = np.ascontiguousarray(lab_ix.reshape(2 * B, 1))

    common = dict(
        loc_tm=loc_tm, tim_tm=tim_tm, lab_ix=lab_ix,
        embed=np.ascontiguousarray(embed_table, dtype=f),
        traj=np.ascontiguousarray(
            np.asarray(traj_table, dtype=f).reshape(2 * TRAJ, H)),
        wih0T=np.ascontiguousarray(np.asarray(w_ih0, dtype=f).T),
        wih1T=np.ascontiguousarray(np.asarray(w_ih1, dtype=f).T),
        whh0T=np.ascontiguousarray(np.asarray(w_hh0, dtype=f).T),
        whh1T=np.ascontiguousarray(np.asarray(w_hh1, dtype=f).T),
    )
    b_ih0 = np.asarray(b_ih0, dtype=f)
    b_hh0 = np.asarray(b_hh0, dtype=f)
    b_ih1 = np.asarray(b_ih1, dtype=f)
    b_hh1 = np.asarray(b_hh1, dtype=f)
    gru_bias_nz = bool(np.any(b_ih0) or np.any(b_hh0) or np.any(b_ih1)
                       or np.any(b_hh1))
    if gru_bias_nz:
        common["gb0"] = np.ascontiguousarray(np.stack([b_ih0, b_hh0]))
        common["gb1"] = np.ascontiguousarray(np.stack([b_ih1, b_hh1]))

    fc_w = np.asarray(fc_w, dtype=f)
    fc_b = np.asarray(fc_b, dtype=f)
    fcb_nz = bool(np.any(fc_b))

    in_maps = []
    for c in range(NCORES):
        m = dict(common)
        wslice = np.concatenate([fc_w[c * VS:(c + 1) * VS], fc_w[OUT:]],
                                axis=0)
        m["fcwT"] = np.ascontiguousarray(wslice.T)
        if fcb_nz:
            bslice = np.concatenate([fc_b[c * VS:(c + 1) * VS], fc_b[OUT:]])
            m["fcb"] = np.ascontiguousarray(bslice.reshape(1, VTOT))
        in_maps.append(m)
    return in_maps, gru_bias_nz, fcb_nz


def _run(in_maps, gru_bias_nz, fcb_nz, trace=False):
    from concourse.bass_utils import run_bass_kernel_spmd
    nc = _get_nc(gru_bias_nz, fcb_nz)
    if trace:
        import sys as _sys
        import types as _types
        try:
            from antenv.axon_hooks import get_axon_ntff_profile_hook  # noqa
        except ImportError:
            from trn_agent_boot.trn_boot import _ntff_profile_via_ctypes
            _h = _ntff_profile_via_ctypes('/opt/axon/libaxon_pjrt.so')
            _m = _types.ModuleType('antenv.axon_hooks')
            _m.get_axon_ntff_profile_hook = lambda: _h
            _m.set_axon_ntff_profile_hook = lambda h: None
            _sys.modules['antenv.axon_hooks'] = _m
    return run_bass_kernel_spmd(nc, in_maps, list(range(NCORES)), trace=trace)


def _assemble(results):
    # device outputs are t-major [R, *] with r = t*32 + b
    loc = np.empty((B, T, OUT), np.float32)
    for c in range(NCORES):
        loc[:, :, c * VS:(c + 1) * VS] = (
            results[c]["out_loc"].reshape(T, B, VS).transpose(1, 0, 2))
    tm = np.ascontiguousarray(
        results[0]["out_tm"].reshape(T, B, TD).transpose(1, 0, 2))
    return loc, tm


def kernel(**inputs):
    in_maps, gru_bias_nz, fcb_nz = _prep_inputs(**inputs)
    res = _run(in_maps, gru_bias_nz, fcb_nz, trace=False)
    return _assemble(res.results)


# revision 37
# speedup vs baseline: 1.7488x; 1.0002x over previous
"""Trainium2 Bass kernel for nn_MetaDiscreteTimeTrajTypeGRUNet.

Strategy (8 NeuronCores, SPMD):
  - GRU (2 layers, T=48, B=32) replicated on all cores (latency-bound, tiny).
    Wavefront schedule: layer 2 runs 1 step behind layer 1; the input
    projections (x@w_ih) are fused into the recurrent PSUM accumulation
    groups, so there is no separate xp precompute.
  - FC + log_softmax vocab-parallel: each core owns 5000 output columns
    (+ every core computes the small 48-col time head; core 0's is used).
  - log_softmax without max-subtraction (logits are O(1) by construction):
    pass 1 (interleaved into the scan as row chunks finish) computes
    sum(exp(x+b)) per row; three staged 2KB AllReduces combine the 8 vocab
    shards (the first two complete during the scan); pass 2 recomputes the
    matmul (weights stay in SBUF) and writes x + b - log(Z) with one
    batched 2.5MB DMA per row chunk.
  - All matmuls in float32r (full PE rate at N>=256).

Row indexing on device is t-major: r = t*32 + b; chunk c = rows 128c..128c+127
(= timesteps 4c..4c+3, all 32 batch elements each). DRAM outputs are written
t-major as contiguous 2.5MB blocks (~3.4x the bandwidth of b-major strided
writes); the host reindexes to [B, T, *] during unsharding.
"""
import numpy as np

B, T, E, TD = 32, 48, 128, 48
H, OUT, TRAJ = 256, 40000, 10
DIN = E + TD          # 176
G = 3 * H             # 768
NCORES = 8
VS = OUT // NCORES    # 5000 vocab columns per core
VTOT = VS + TD        # 5048 fc columns per core (vocab slice + time head)
R = B * T             # 1536
RC = R // 12          # noqa: E501  (unused alias guard)
RC = R // 128         # 12 row chunks
NCH = 10              # vocab N-chunks per row chunk
NW = VS // NCH        # 500
NGRP = 3              # allreduce groups (4 row chunks each)

_cache = {}


def _build(gru_bias_nz: bool, fcb_nz: bool):
    import concourse.bass as bass
    import concourse.mybir as mybir
    import concourse.tile as tile
    import concourse.bacc as bacc
    from concourse.masks import make_identity

    f32 = mybir.dt.float32
    f32r = mybir.dt.float32r
    i32 = mybir.dt.int32
    AF = mybir.ActivationFunctionType
    ALU = mybir.AluOpType
    AX = mybir.AxisListType

    nc = bacc.Bacc("TRN2", target_bir_lowering=False, debug=False,
                   num_devices=NCORES)

    # ---------------- I/O ----------------
    loc_tm = nc.dram_tensor("loc_tm", [128, RC], i32, kind="ExternalInput")
    tim_tm = nc.dram_tensor("tim_tm", [128, RC], i32, kind="ExternalInput")
    lab_ix = nc.dram_tensor("lab_ix", [2 * B, 1], i32, kind="ExternalInput")
    embed = nc.dram_tensor("embed", [OUT, E], f32, kind="ExternalInput")
    traj = nc.dram_tensor("traj", [2 * TRAJ, H], f32, kind="ExternalInput")
    wih0T_d = nc.dram_tensor("wih0T", [DIN, G], f32, kind="ExternalInput")
    wih1T_d = nc.dram_tensor("wih1T", [H, G], f32, kind="ExternalInput")
    whh0T_d = nc.dram_tensor("whh0T", [H, G], f32, kind="ExternalInput")
    whh1T_d = nc.dram_tensor("whh1T", [H, G], f32, kind="ExternalInput")
    fcwT_d = nc.dram_tensor("fcwT", [H, VTOT], f32, kind="ExternalInput")
    if gru_bias_nz:
        gb0_d = nc.dram_tensor("gb0", [2, G], f32, kind="ExternalInput")
        gb1_d = nc.dram_tensor("gb1", [2, G], f32, kind="ExternalInput")
    if fcb_nz:
        fcb_d = nc.dram_tensor("fcb", [1, VTOT], f32, kind="ExternalInput")

    # outputs are written t-major (row r = t*32+b, contiguous 2.5MB blocks
    # per row chunk — ~3.4x the DMA bandwidth of b-major strided writes);
    # the host reindexes to [B, T, *]
    out_loc = nc.dram_tensor("out_loc", [R, VS], f32, kind="ExternalOutput")
    out_tm = nc.dram_tensor("out_tm", [R, TD], f32, kind="ExternalOutput")

    with tile.TileContext(nc) as tc:
        with (
            tc.tile_pool(name="const", bufs=1) as cp,
            tc.tile_pool(name="dram", bufs=1, space="DRAM") as dp,
        ):
            # ---------- constants ----------
            idn = cp.tile([128, 128], f32, tag="idn", name="idn")
            make_identity(nc, idn[:])

            iota_i = cp.tile([128, TD], i32, tag="iota_i", name="iota_i")
            nc.gpsimd.iota(iota_i[:], pattern=[[1, TD]], base=0,
                           channel_multiplier=0)
            iota_f = cp.tile([128, TD], f32, tag="iota_f", name="iota_f")
            nc.vector.tensor_copy(iota_f[:], iota_i[:])

            loc_sb = cp.tile([128, RC], i32, tag="loc_sb", name="loc_sb")
            nc.sync.dma_start(loc_sb[:], loc_tm[:])
            tim_i = cp.tile([128, RC], i32, tag="tim_i", name="tim_i")
            nc.sync.dma_start(tim_i[:], tim_tm[:])
            tim_f = cp.tile([128, RC], f32, tag="tim_f", name="tim_f")
            nc.vector.tensor_copy(tim_f[:], tim_i[:])

            # h0 gather early (overlaps the weight loads below)
            h0b = {}
            for l, li in ((1, 0), (2, 1)):
                ls = cp.tile([B, 1], i32, tag=f"lab{l}", name=f"lab{l}")
                nc.sync.dma_start(ls[:], lab_ix[32 * li:32 * li + 32, :])
                hb = cp.tile([B, H], f32, tag=f"h0b{l}", name=f"h0b{l}")
                nc.gpsimd.indirect_dma_start(
                    out=hb[:], out_offset=None, in_=traj[:],
                    in_offset=bass.IndirectOffsetOnAxis(ap=ls[:, :1], axis=0),
                )
                h0b[l] = hb

            # weights -> SBUF as f32r via staging + rounding copy
            # (scan weights first; the big fc slabs last)
            with tc.tile_pool(name="stage", bufs=2) as stp:
                def load_w(dram_h, r0, r1, cols, tag):
                    st = stp.tile([r1 - r0, cols], f32, tag="wstage",
                                  name="wstage")
                    nc.sync.dma_start(st[:], dram_h[r0:r1, :])
                    t = cp.tile([r1 - r0, cols], f32r, tag=tag, name=tag)
                    nc.vector.tensor_copy(t[:], st[:])
                    return t

                wih0a = load_w(wih0T_d, 0, 128, G, "wih0a")
                wih0b = load_w(wih0T_d, 128, DIN, G, "wih0b")
                wih1a = load_w(wih1T_d, 0, 128, G, "wih1a")
                wih1b = load_w(wih1T_d, 128, H, G, "wih1b")
                whh = {
                    1: (load_w(whh0T_d, 0, 128, G, "whh0a"),
                        load_w(whh0T_d, 128, H, G, "whh0b")),
                    2: (load_w(whh1T_d, 0, 128, G, "whh1a"),
                        load_w(whh1T_d, 128, H, G, "whh1b")),
                }
                fcwa = load_w(fcwT_d, 0, 128, VTOT, "fcwa")
                fcwb = load_w(fcwT_d, 128, H, VTOT, "fcwb")

            # transposed y storage, two K-halves side by side:
            # [:, 0:R] = K rows 0:128, [:, R:2R] = K rows 128:256
            y1T = cp.tile([128, 2 * R], f32r, tag="y1T", name="y1T")
            yrT = cp.tile([128, 2 * R], f32r, tag="yrT", name="yrT")

            # optional fc bias materialization via ones-matmul broadcast
            fcb_all = None
            if fcb_nz:
                ones_s = cp.tile([1, 128], f32, tag="ones_s", name="ones_s")
                nc.vector.memset(ones_s[:1, :], 1.0)
                ones_r = cp.tile([1, 128], f32r, tag="ones_r", name="ones_r")
                nc.vector.tensor_copy(ones_r[:1, :], ones_s[:1, :])
                with tc.tile_pool(name="bps", bufs=1, space="PSUM") as bps:
                    row_s = cp.tile([1, VTOT], f32, tag="fcb_row_s",
                                    name="fcb_row_s")
                    nc.sync.dma_start(row_s[:1, :], fcb_d[:])
                    row = cp.tile([1, VTOT], f32r, tag="fcb_row",
                                  name="fcb_row")
                    nc.vector.tensor_copy(row[:1, :], row_s[:1, :])
                    fcb_all = cp.tile([128, VTOT], f32, tag="fcb_full",
                                      name="fcb_full")
                    for n0 in range(0, VTOT, 512):
                        n1 = min(n0 + 512, VTOT)
                        pb = bps.tile([128, 512], f32, tag="bps", name="bps")
                        nc.tensor.matmul(pb[:, 0:n1 - n0], ones_r[:1, :],
                                         row[:1, n0:n1], start=True, stop=True)
                        nc.vector.tensor_copy(fcb_all[:, n0:n1],
                                              pb[:, 0:n1 - n0])

            # GRU bias rows (general path) as K=1 rank-1 matmul contributions
            brz = {}
            bhn = {}
            bxn = {}
            if gru_bias_nz:
                ones32 = cp.tile([1, 32], f32, tag="ones32", name="ones32")
                nc.vector.memset(ones32[:1, :], 1.0)
                ones32r = cp.tile([1, 32], f32r, tag="ones32r", name="ones32r")
                nc.vector.tensor_copy(ones32r[:1, :], ones32[:1, :])
                for l, gbd in ((1, gb0_d), (2, gb1_d)):
                    bi = cp.tile([1, G], f32, tag=f"bi{l}", name=f"bi{l}")
                    bh = cp.tile([1, G], f32, tag=f"bh{l}", name=f"bh{l}")
                    nc.sync.dma_start(bi[:1, :], gbd[0:1, :])
                    nc.sync.dma_start(bh[:1, :], gbd[1:2, :])
                    bs = cp.tile([1, 512], f32, tag=f"brzs{l}",
                                 name=f"brzs{l}")
                    nc.vector.tensor_tensor(out=bs[:1, :], in0=bi[:1, 0:512],
                                            in1=bh[:1, 0:512], op=ALU.add)
                    br = cp.tile([1, 512], f32r, tag=f"brz{l}", name=f"brz{l}")
                    nc.vector.tensor_copy(br[:1, :], bs[:1, :])
                    bn1 = cp.tile([1, 256], f32r, tag=f"bhn{l}",
                                  name=f"bhn{l}")
                    nc.vector.tensor_copy(bn1[:1, :], bh[:1, 512:G])
                    bn2 = cp.tile([1, 256], f32r, tag=f"bxn{l}",
                                  name=f"bxn{l}")
                    nc.vector.tensor_copy(bn2[:1, :], bi[:1, 512:G])
                    brz[l], bhn[l], bxn[l] = br, bn1, bn2

            # ---------- scan + interleaved FC pass 1 ----------
            with (
                tc.tile_pool(name="scan2", bufs=2) as sp2,
                tc.tile_pool(name="scan3", bufs=3) as sp3,
                tc.tile_pool(name="fcp1", bufs=1) as fc1,
                tc.tile_pool(name="fcp3", bufs=3) as fc3,
                tc.tile_pool(name="ps_fc", bufs=2, space="PSUM") as fp,
                tc.tile_pool(name="fout_p", bufs=2) as fop,
            ):
                accs = fc1.tile([128, RC], f32, tag="accs", name="accs")
                nlz = fc1.tile([128, RC], f32, tag="nlz", name="nlz")
                rsums = fc1.tile([128, RC], f32, tag="rsums", name="rsums")
                tm_all = fc1.tile([128, RC * TD], f32, tag="tm_all",
                                  name="tm_all")
                cc_in = [dp.tile([128, 4], f32, name=f"cc_in{g}")
                         for g in range(NGRP)]
                cc_out = [dp.tile([128, 4], f32, name=f"cc_out{g}")
                          for g in range(NGRP)]

                scan_ps = tc.tile_pool(name="ps_hp", bufs=2, space="PSUM")
                psh = scan_ps.__enter__()
                scan_ps2 = tc.tile_pool(name="ps_xn", bufs=1, space="PSUM")
                psn = scan_ps2.__enter__()
                scan_ps3 = tc.tile_pool(name="ps_tp", bufs=1, space="PSUM")
                pst = scan_ps3.__enter__()

                def tp_tile():
                    return pst.tile([128, 256], f32, tag="tp", name="tp")

                # h0 transposes -> f32r lhsT tiles
                h0T = {}
                for l in (1, 2):
                    pt = tp_tile()
                    nc.tensor.transpose(pt[:, 0:32], h0b[l][:, 0:128],
                                        idn[0:32, 0:32])
                    nc.tensor.transpose(pt[:, 32:64], h0b[l][:, 128:256],
                                        idn[0:32, 0:32])
                    ht = cp.tile([128, 64], f32r, tag=f"h0T{l}",
                                 name=f"h0T{l}")
                    nc.vector.tensor_copy(ht[:], pt[:, 0:64])
                    h0T[l] = ht

                def make_xcatT(c):
                    xc = sp2.tile([128, DIN], f32, tag="xcat", name="xcat")
                    nc.gpsimd.indirect_dma_start(
                        out=xc[:, 0:E], out_offset=None, in_=embed[:],
                        in_offset=bass.IndirectOffsetOnAxis(
                            ap=loc_sb[:, c:c + 1], axis=0),
                    )
                    nc.vector.tensor_scalar(
                        out=xc[:, E:DIN], in0=iota_f[:, 0:TD],
                        scalar1=tim_f[:, c:c + 1], scalar2=None,
                        op0=ALU.is_equal,
                    )
                    pt = tp_tile()
                    nc.tensor.transpose(pt[:, 0:128], xc[:, 0:128],
                                        idn[0:128, 0:128])
                    nc.tensor.transpose(pt[0:48, 128:256], xc[:, 128:DIN],
                                        idn[0:128, 0:128])
                    xa = sp3.tile([128, 128], f32r, tag="xcatTa",
                                  name="xcatTa")
                    xb = sp3.tile([48, 128], f32r, tag="xcatTb", name="xcatTb")
                    nc.vector.tensor_copy(xa[:], pt[:, 0:128])
                    nc.vector.tensor_copy(xb[:], pt[0:48, 128:256])
                    return xa, xb

                def gru_step(l, t, xA, xB, wxa, wxb, hprev_b, hTa, hTb):
                    """One GRU step, [32, *] batch-on-partition layout."""
                    wa, wb = whh[l]
                    hp = psh.tile([32, G], f32, tag="hp", name="hp")
                    # r+z: one N=512 accumulation group (x-proj + bias + h)
                    nc.tensor.matmul(hp[:, 0:512], xA, wxa[:, 0:512],
                                     start=True, stop=False)
                    nc.tensor.matmul(hp[:, 0:512], xB, wxb[:, 0:512],
                                     start=False, stop=False)
                    if gru_bias_nz:
                        nc.tensor.matmul(hp[:, 0:512], ones32r[:1, :],
                                         brz[l][:1, :], start=False,
                                         stop=False)
                    nc.tensor.matmul(hp[:, 0:512], hTa, wa[:, 0:512],
                                     start=False, stop=False)
                    nc.tensor.matmul(hp[:, 0:512], hTb, wb[:, 0:512],
                                     start=False, stop=True)
                    # hn
                    nc.tensor.matmul(hp[:, 512:G], hTa, wa[:, 512:G],
                                     start=True, stop=False)
                    nc.tensor.matmul(hp[:, 512:G], hTb, wb[:, 512:G],
                                     start=False, stop=not gru_bias_nz)
                    if gru_bias_nz:
                        nc.tensor.matmul(hp[:, 512:G], ones32r[:1, :],
                                         bhn[l][:1, :], start=False, stop=True)
                    # xn
                    xn = psn.tile([32, 256], f32, tag="xn", name="xn")
                    nc.tensor.matmul(xn[:], xA, wxa[:, 512:G],
                                     start=True, stop=False)
                    nc.tensor.matmul(xn[:], xB, wxb[:, 512:G],
                                     start=False, stop=not gru_bias_nz)
                    if gru_bias_nz:
                        nc.tensor.matmul(xn[:], ones32r[:1, :], bxn[l][:1, :],
                                         start=False, stop=True)
                    s = sp2.tile([32, G], f32, tag=f"s{l}", name=f"s{l}")
                    g = sp2.tile([32, G], f32, tag=f"g{l}", name=f"g{l}")
                    # r,z: sigmoid(x) = 0.5*tanh(x/2) + 0.5 — keeps the whole
                    # kernel inside the Tanh+Exp ACT table set (no reloads)
                    nc.scalar.activation(g[:, 0:512], hp[:, 0:512], AF.Tanh,
                                         scale=0.5)
                    nc.vector.tensor_scalar(
                        out=g[:, 0:512], in0=g[:, 0:512], scalar1=0.5,
                        scalar2=0.5, op0=ALU.mult, op1=ALU.add)
                    # n = tanh(xn + r*hn)
                    nc.vector.tensor_tensor(out=s[:, 512:G], in0=g[:, 0:256],
                                            in1=hp[:, 512:G], op=ALU.mult)
                    nc.vector.tensor_tensor(out=s[:, 512:G], in0=s[:, 512:G],
                                            in1=xn[:], op=ALU.add)
                    nc.scalar.activation(g[:, 512:G], s[:, 512:G], AF.Tanh)
                    # h_new = n + z*(h_prev - n)
                    nc.vector.tensor_tensor(out=s[:, 0:256], in0=hprev_b,
                                            in1=g[:, 512:G], op=ALU.subtract)
                    nc.vector.tensor_tensor(out=s[:, 256:512],
                                            in0=g[:, 256:512],
                                            in1=s[:, 0:256], op=ALU.mult)
                    hn = sp2.tile([32, H], f32, tag=f"h{l}", name=f"h{l}")
                    nc.vector.tensor_tensor(out=hn[:], in0=g[:, 512:G],
                                            in1=s[:, 256:512], op=ALU.add)
                    # transpose h_new for the next step's lhsT (+ storage)
                    pt = tp_tile()
                    nc.tensor.transpose(pt[:, 0:32], hn[:, 0:128],
                                        idn[0:32, 0:32])
                    nc.tensor.transpose(pt[:, 32:64], hn[:, 128:256],
                                        idn[0:32, 0:32])
                    pt2 = pt[:, 0:64].rearrange("p (k r) -> p k r", k=2)
                    if l == 1:
                        dst = y1T.rearrange("p (k r) -> p k r",
                                            k=2)[:, :, 32 * t:32 * t + 32]
                        nc.vector.tensor_copy(dst, pt2)
                        nTa = y1T[:, 32 * t:32 * t + 32]
                        nTb = y1T[:, R + 32 * t:R + 32 * t + 32]
                    else:
                        ht = sp2.tile([128, 64], f32r, tag="h2T", name="h2T")
                        nc.vector.tensor_copy(ht[:], pt[:, 0:64])
                        rdst = yrT.rearrange("p (k r) -> p k r",
                                             k=2)[:, :, 32 * t:32 * t + 32]
                        nc.scalar.activation(rdst, pt2, AF.Relu)
                        nTa, nTb = ht[:, 0:32], ht[:, 32:64]
                    return hn, nTa, nTb

                # FC pass-1 quarter-chunks: compute sum(exp(x+b)) per row.
                # (The time head and all Ln work run post-scan so the scan
                # window stays inside one ACT table set.)
                NPARTS = ((0, 1, 2), (3, 4, 5), (6, 7), (8, 9))
                accn_tiles = {}

                def fc_mm(pp, c, n0, n1):
                    nc.tensor.matmul(pp[:, 0:n1 - n0],
                                     yrT[:, 128 * c:128 * c + 128],
                                     fcwa[:, n0:n1], start=True, stop=False)
                    nc.tensor.matmul(pp[:, 0:n1 - n0],
                                     yrT[:, R + 128 * c:R + 128 * c + 128],
                                     fcwb[:, n0:n1], start=False, stop=True)

                def pass1_part(c, part):
                    if part == 0:
                        accn_tiles[c] = fc3.tile([128, NCH], f32, tag="accn",
                                                 name="accn")
                    accn = accn_tiles[c]
                    for n in NPARTS[part]:
                        pp = fp.tile([128, 500], f32, tag="fc", name="fc")
                        fc_mm(pp, c, NW * n, NW * (n + 1))
                        es = fc3.tile([128, NW], f32, tag="exps", name="exps")
                        if fcb_nz:
                            nc.vector.tensor_tensor(
                                out=es[:], in0=pp[:, 0:NW],
                                in1=fcb_all[:, NW * n:NW * (n + 1)],
                                op=ALU.add)
                            nc.scalar.activation(
                                es[:], es[:], AF.Exp,
                                accum_out=accn[:, n:n + 1])
                        else:
                            nc.scalar.activation(
                                es[:], pp[:, 0:NW], AF.Exp,
                                accum_out=accn[:, n:n + 1])
                    if part == 3:
                        nc.vector.tensor_reduce(
                            accs[:, c:c + 1], accn[:], axis=AX.X, op=ALU.add)

                def launch_ar(g):
                    nc.sync.dma_start(cc_in[g][:], accs[:, 4 * g:4 * g + 4])
                    nc.gpsimd.collective_compute(
                        "AllReduce", ALU.add,
                        replica_groups=[list(range(NCORES))],
                        ins=[cc_in[g].opt()], outs=[cc_out[g].opt()],
                    )

                def collect_group(g):
                    nc.sync.dma_start(rsums[:, 4 * g:4 * g + 4],
                                      cc_out[g][:])
                    nc.scalar.activation(nlz[:, 4 * g:4 * g + 4],
                                         rsums[:, 4 * g:4 * g + 4], AF.Ln)
                    nc.vector.tensor_scalar_mul(nlz[:, 4 * g:4 * g + 4],
                                                nlz[:, 4 * g:4 * g + 4], -1.0)

                def pass2_chunk(c, fcp):
                    fo = fop.tile([128, VS], f32, tag="fout", name="fout")
                    for n in range(NCH):
                        pp = fcp.tile([128, 500], f32, tag="fc", name="fc")
                        fc_mm(pp, c, NW * n, NW * (n + 1))
                        fs_ = fo[:, NW * n:NW * (n + 1)]
                        if fcb_nz:
                            nc.vector.scalar_tensor_tensor(
                                out=fs_, in0=pp[:, 0:NW],
                                scalar=nlz[:, c:c + 1],
                                in1=fcb_all[:, NW * n:NW * (n + 1)],
                                op0=ALU.add, op1=ALU.add)
                        elif n % 2 == 0:
                            nc.vector.tensor_scalar(
                                out=fs_, in0=pp[:, 0:NW],
                                scalar1=nlz[:, c:c + 1],
                                scalar2=None, op0=ALU.add)
                        else:
                            nc.scalar.activation(fs_, pp[:, 0:NW],
                                                 AF.Identity,
                                                 bias=nlz[:, c:c + 1])
                    nc.sync.dma_start(
                        bass.AP(out_loc, 128 * c * VS, [[VS, 128], [1, VS]]),
                        fo[:])

                # ---------- the scan loop ----------
                xcatT = {0: make_xcatT(0), 1: make_xcatT(1)}
                h1b, h2b = h0b[1][:, :], h0b[2][:, :]
                h1Ta, h1Tb = h0T[1][:, 0:32], h0T[1][:, 32:64]
                h2Ta, h2Tb = h0T[2][:, 0:32], h0T[2][:, 32:64]

                def l2_step(t2, h2b, h2Ta, h2Tb):
                    return gru_step(2, t2,
                                    y1T[:, 32 * t2:32 * t2 + 32],
                                    y1T[:, R + 32 * t2:R + 32 * t2 + 32],
                                    wih1a, wih1b, h2b, h2Ta, h2Tb)

                if True:
                    for t in range(T):
                        if t % 4 == 0 and t // 4 + 2 < RC:
                            xcatT[t // 4 + 2] = make_xcatT(t // 4 + 2)
                        xa, xb = xcatT[t // 4]
                        tau = 32 * (t % 4)
                        hn, h1Ta, h1Tb = gru_step(
                            1, t, xa[:, tau:tau + 32], xb[:, tau:tau + 32],
                            wih0a, wih0b, h1b, h1Ta, h1Tb)
                        h1b = hn[:]
                        if t >= 1:
                            hn2, h2Ta, h2Tb = l2_step(t - 1, h2b, h2Ta, h2Tb)
                            h2b = hn2[:]
                        # interleave FC pass 1 for finished row chunks:
                        # chunk c is done after layer-2 step 4c+3 (wave 4c+4)
                        if t >= 4:
                            c, part = (t - 4) // 4, (t - 4) % 4
                            if c < RC - 1:
                                pass1_part(c, part)
                        if t == 20:
                            launch_ar(0)
                        if t == 36:
                            launch_ar(1)
                        # stream the first pass-2 chunks into the late scan
                        # (their AllReduce finished ~15 waves after launch)
                        if t == 38:
                            collect_group(0)
                        if t in (39, 42, 45):
                            pass2_chunk((t - 39) // 3, fp)

                    hn2, h2Ta, h2Tb = l2_step(T - 1, h2b, h2Ta, h2Tb)
                    for part in range(4):
                        pass1_part(RC - 1, part)
                    scan_ps3.__exit__(None, None, None)
                    scan_ps2.__exit__(None, None, None)
                    scan_ps.__exit__(None, None, None)
                    launch_ar(2)
                    collect_group(1)

                    # time head: local 48-wide log-softmax (post-scan; needs
                    # the Ln table set)
                    for c in range(RC):
                        pp = fp.tile([128, 500], f32, tag="fc", name="fc")
                        fc_mm(pp, c, VS, VTOT)
                        tme = fc3.tile([128, TD], f32, tag="tme", name="tme")
                        tma = fc3.tile([128, 1], f32, tag="tma", name="tma")
                        if fcb_nz:
                            nc.vector.tensor_tensor(
                                out=tme[:], in0=pp[:, 0:TD],
                                in1=fcb_all[:, VS:VTOT], op=ALU.add)
                            nc.scalar.activation(tme[:], tme[:], AF.Exp,
                                                 accum_out=tma[:])
                        else:
                            nc.scalar.activation(tme[:], pp[:, 0:TD], AF.Exp,
                                                 accum_out=tma[:])
                        tml = fc3.tile([128, 1], f32, tag="tml", name="tml")
                        nc.scalar.activation(tml[:], tma[:], AF.Ln)
                        nc.vector.tensor_scalar_mul(tml[:], tml[:], -1.0)
                        tmo = tm_all[:, TD * c:TD * (c + 1)]
                        if fcb_nz:
                            nc.vector.scalar_tensor_tensor(
                                out=tmo, in0=pp[:, 0:TD], scalar=tml[:, :1],
                                in1=fcb_all[:, VS:VTOT],
                                op0=ALU.add, op1=ALU.add)
                        else:
                            nc.vector.tensor_scalar(
                                out=tmo, in0=pp[:, 0:TD], scalar1=tml[:, :1],
                                scalar2=None, op0=ALU.add)

                    # remaining pass-2 chunks with a deep psum pipeline
                    with tc.tile_pool(name="ps_fc2", bufs=5,
                                      space="PSUM") as fp2:
                        for c in range(3, RC):
                            if c == 8:
                                collect_group(2)
                            pass2_chunk(c, fp2)

                    # single DMA for the whole time-head output (t-major)
                    nc.sync.dma_start(
                        bass.AP(out_tm, 0, [[TD, 128], [128 * TD, RC], [1, TD]]),
                        tm_all[:])

    nc.compile()
    return nc


def _get_nc(gru_bias_nz, fcb_nz):
    key = (gru_bias_nz, fcb_nz)
    if key not in _cache:
        _cache[key] = _build(*key)
    return _cache[key]


def _prep_inputs(locations, times, labels, embed_table, traj_table, fc_w, fc_b,
                 w_ih0, w_hh0, b_ih0, b_hh0, w_ih1, w_hh1, b_ih1, b_hh1):
    f = np.float32
    locations = np.asarray(locations)
    times = np.asarray(times)
    labels = np.asarray(labels)
    # t-major row layout: r = t*32 + b; [128, RC] with [p, c] = row 128c+p
    loc_tm = np.ascontiguousarray(
        locations.T.reshape(RC, 128).T).astype(np.int32)
    tim_tm = np.ascontiguousarray(
        times.T.reshape(RC, 128).T).astype(np.int32)
    # h0 strip-gather indices (torch .view(L, -1, H) semantics):
    # h0[l, b] = traj_table.view(20, 256)[2*labels[16l + b//2] + b%2]
    p = np.arange(2 * B)
    l_, b_ = p // B, p % B
    lab_ix = (2 * labels[(B // 2) * l_ + b_ // 2] + b_ % 2).astype(np.int32)
    lab_ix = np.ascontiguousarray(lab_ix.reshape(2 * B, 1))

    common = dict(
        loc_tm=loc_tm, tim_tm=tim_tm, lab_ix=lab_ix,
        embed=np.ascontiguousarray(embed_table, dtype=f),
        traj=np.ascontiguousarray(
            np.asarray(traj_table, dtype=f).reshape(2 * TRAJ, H)),
        wih0T=np.ascontiguousarray(np.asarray(w_ih0, dtype=f).T),
        wih1T=np.ascontiguousarray(np.asarray(w_ih1, dtype=f).T),
        whh0T=np.ascontiguousarray(np.asarray(w_hh0, dtype=f).T),
        whh1T=np.ascontiguousarray(np.asarray(w_hh1, dtype=f).T),
    )
    b_ih0 = np.asarray(b_ih0, dtype=f)
    b_hh0 = np.asarray(b_hh0, dtype=f)
    b_ih1 = np.asarray(b_ih1, dtype=f)
    b_hh1 = np.asarray(b_hh1, dtype=f)
    gru_bias_nz = bool(np.any(b_ih0) or np.any(b_hh0) or np.any(b_ih1)
                       or np.any(b_hh1))
    if gru_bias_nz:
        common["gb0"] = np.ascontiguousarray(np.stack([b_ih0, b_hh0]))
        common["gb1"] = np.ascontiguousarray(np.stack([b_ih1, b_hh1]))

    fc_w = np.asarray(fc_w, dtype=f)
    fc_b = np.asarray(fc_b, dtype=f)
    fcb_nz = bool(np.any(fc_b))

    in_maps = []
    for c in range(NCORES):
        m = dict(common)
        wslice = np.concatenate([fc_w[c * VS:(c + 1) * VS], fc_w[OUT:]],
                                axis=0)
        m["fcwT"] = np.ascontiguousarray(wslice.T)
        if fcb_nz:
            bslice = np.concatenate([fc_b[c * VS:(c + 1) * VS], fc_b[OUT:]])
            m["fcb"] = np.ascontiguousarray(bslice.reshape(1, VTOT))
        in_maps.append(m)
    return in_maps, gru_bias_nz, fcb_nz


def _run(in_maps, gru_bias_nz, fcb_nz, trace=False):
    from concourse.bass_utils import run_bass_kernel_spmd
    nc = _get_nc(gru_bias_nz, fcb_nz)
    if trace:
        import sys as _sys
        import types as _types
        try:
            from antenv.axon_hooks import get_axon_ntff_profile_hook  # noqa
        except ImportError:
            from trn_agent_boot.trn_boot import _ntff_profile_via_ctypes
            _h = _ntff_profile_via_ctypes('/opt/axon/libaxon_pjrt.so')
            _m = _types.ModuleType('antenv.axon_hooks')
            _m.get_axon_ntff_profile_hook = lambda: _h
            _m.set_axon_ntff_profile_hook = lambda h: None
            _sys.modules['antenv.axon_hooks'] = _m
    return run_bass_kernel_spmd(nc, in_maps, list(range(NCORES)), trace=trace)


def _assemble(results):
    # device outputs are t-major [R, *] with r = t*32 + b
    loc = np.empty((B, T, OUT), np.float32)
    for c in range(NCORES):
        loc[:, :, c * VS:(c + 1) * VS] = (
            results[c]["out_loc"].reshape(T, B, VS).transpose(1, 0, 2))
    tm = np.ascontiguousarray(
        results[0]["out_tm"].reshape(T, B, TD).transpose(1, 0, 2))
    return loc, tm


def kernel(**inputs):
    in_maps, gru_bias_nz, fcb_nz = _prep_inputs(**inputs)
    res = _run(in_maps, gru_bias_nz, fcb_nz, trace=False)
    return _assemble(res.results)
